# revision 61
# baseline (speedup 1.0000x reference)
"""BitMGQA forward for Trainium2, 8-core SPMD Bass/Tile kernel.

Sharding: the B*S = 4096 tokens are split into 8 slices of 512 (cores 0-3 =
batch 0, cores 4-7 = batch 1). Each core quantizes + projects its own token
slice for Q/K/V; the per-batch K/V (plus their per-token output scales) are
AllGathered across each 4-core batch group in one merged collective;
attention, layernorm and the output projection are then fully local to each
core's 512 query tokens.

BitLinear structure is exploited: activation quantization produces integers in
[-128, 127] and weight binarization produces +-1, both exactly representable
in bf16, so every projection matmul runs on the PE in bf16 with fp32 PSUM
accumulation exactly; rmsnorm / quant / weight scales fold into one per-token
fp32 scale applied to the matmul output (or, for K, into the softmax exp's
per-key scale operand).

Attention runs "AV-transposed": scores stay key-major ([key, token] psum,
exp'd in place with the per-key scale), and AV contracts keys with V as the
128-wide stationary so the moving dim is 512 tokens (big frees, few
instructions). Softmax denominators come from an all-ones stationary matmul
over the same probs. The [head_dim, token] attention output is divided by the
denominator, transposed back token-major on the PE (fp32-exact) with Pool
draining PSUM, and the layernorm/quant/out-proj tail runs token-major.

Scheduling: only the K/V path runs before the AllGather; Q quant/projection,
q/o weight prep and the *local* quarter of scores+exp fill the collective
window.
"""
import contextlib

import numpy as np

import concourse.bass as bass
import concourse.mybir as mybir
import concourse.tile as tile

dt = mybir.dt
AF = mybir.ActivationFunctionType
ALU = mybir.AluOpType
AX = mybir.AxisListType

# problem dims (hardcoded per contract)
B, S, D = 2, 2048, 1024
N_CORES = 8
GRP = 4                    # cores per batch group
TPC = (B * S) // N_CORES   # 512 tokens per core
P = 128
NTC = TPC // P             # 4 token tiles per core
DC = D // P                # 8 contraction chunks
KVE = 256
QH, KVH, HD = 8, 2, 128
NKC = S // P               # 16 key chunks per batch
LKC = TPC // P             # 4 key chunks produced locally
MAGIC = 12582912.0         # 1.5 * 2**23: (x + MAGIC) - MAGIC == rint(x)
INV_SQRT_HD = float(HD) ** -0.5
LN_EPS = 1e-5
RMS_EPS = 1e-6

_BUILT = {}


def _split_multiwaits(nc, max_waits=1):
    """The pinned walrus rejects >1 sync-wait per instruction ("Too many sync
    wait commands"). Split extras onto single-wait NoOps inserted before the
    offending instruction on the same engine (sequencer stalls in order, so
    semantics are identical)."""
    for f in nc.m.functions:
        for bb in f.blocks:
            insts = list(bb.instructions)
            if not any(
                i.sync_info is not None and len(i.sync_info.on_wait) > max_waits
                for i in insts
            ):
                continue
            new_insts = []
            for ins in insts:
                si = ins.sync_info
                if si is not None and len(si.on_wait) > max_waits:
                    waits = list(si.on_wait)
                    for k, w in enumerate(waits[:-max_waits]):
                        new_insts.append(mybir.InstNoOp(
                            name=f"{ins.name}-wsplit{k}",
                            engine=ins.engine,
                            sync_info=mybir.SyncInfo(on_wait=[w], on_update=[]),
                        ))
                    ins.sync_info = mybir.SyncInfo(
                        on_wait=waits[-max_waits:], on_update=list(si.on_update)
                    )
                new_insts.append(ins)
            bb.instructions = new_insts


class _Emit:
    """Per-build emission state."""

    def __init__(self, nc, tc_, ctx):
        self.nc = nc
        self.tc = tc_
        f32 = dt.float32
        self.small = ctx.enter_context(tc_.tile_pool(name="small", bufs=1))
        self.persist = ctx.enter_context(tc_.tile_pool(name="persist", bufs=1))
        self.pipe = ctx.enter_context(tc_.tile_pool(name="pipe", bufs=3))
        self.live = ctx.enter_context(tc_.tile_pool(name="live", bufs=1))
        self.dram = ctx.enter_context(
            tc_.tile_pool(name="dram", bufs=1, space="DRAM"))
        self.ones = self.small.tile([P, P], f32, tag="ones128", name="ones")
        nc.vector.memset(self.ones[:], 1.0)
        self.ones_bf = self.small.tile([P, P], dt.bfloat16, tag="ones128b",
                                       name="ones_bf")
        nc.vector.memset(self.ones_bf[:], 1.0)
        self.eps_rms = self.small.tile([P, 1], f32, tag="eps_rms", name="eps_rms")
        nc.vector.memset(self.eps_rms[:], float(D * RMS_EPS))
        self.eps_ln = self.small.tile([P, 1], f32, tag="eps_ln", name="eps_ln")
        nc.vector.memset(self.eps_ln[:], LN_EPS)
        # identity for PE transposes, loaded from an external input (the
        # gpsimd affine_select path is rejected by this walrus build);
        # build_nc fills these right after construction
        self.ident = self.small.tile([P, P], f32, tag="ident", name="ident")
        self.ident_bf = self.small.tile([P, P], dt.bfloat16, tag="ident_bf",
                                        name="ident_bf")

    # ---- helpers -------------------------------------------------------
    def weight_prep(self, w_dram, n_oc, swT, name, psred, wpool=None,
                    load_eng=None, pe_t=False, wait_ms=None, drain="act"):
        """sign(w)^T into swT [P, n_oc, DC, P] bf16; returns mean|w| as a
        [P, 1] fp32 column replicated across partitions. Loads weight rows
        two 128-row chunks at a time to halve DMA count. pe_t: transpose on
        the PE (Pool drains PSUM) instead of the DMA xbar — required after
        the collective is issued, since DmaTranspose serializes with it."""
        nc = self.nc
        wpool = wpool or self.wpool
        npair = n_oc // 2
        abscol = self.small.tile(
            [P, npair], dt.float32, tag=f"abscol_{name}", name=f"abscol_{name}")
        wv = w_dram.rearrange("(oc p) d -> p oc d", p=P)
        for pr in range(npair):
            wt = wpool.tile([P, 2, D], dt.float32, tag="wtile", bufs=3,
                            name="wt")
            with self.tc.tile_wait_until(wait_ms or 0, enable=wait_ms is not None):
                (load_eng or nc.sync).dma_start(wt[:], wv[:, 2 * pr:2 * pr + 2])
            sw = wpool.tile([P, 2, D], dt.bfloat16, tag="swtile", bufs=2,
                            name="sw")
            nc.scalar.sign(sw[:], wt[:])
            nc.vector.tensor_reduce(
                abscol[:, pr:pr + 1], wt[:], AX.XY, ALU.add,
                apply_absolute_value=True,
            )
            if pe_t:
                pswt = psred.tile([P, 2, DC, P], dt.bfloat16, tag="wtp",
                                  bufs=1, name="pswt")
                for i in range(2):
                    for dc_ in range(DC):
                        nc.tensor.matmul(
                            pswt[:, i, dc_, :], sw[:, i, dc_ * P:(dc_ + 1) * P],
                            self.ident_bf[:], is_transpose=True,
                            start=True, stop=True)
                    if drain == "act":
                        nc.scalar.copy(swT[:, 2 * pr + i], pswt[:, i])
                    else:
                        nc.vector.tensor_copy(swT[:, 2 * pr + i], pswt[:, i])
            else:
                for i in range(2):
                    nc.sync.dma_start_transpose(swT[:, 2 * pr + i], sw[:, i])
        rowtot = self.small.tile(
            [P, 1], dt.float32, tag=f"rowtot_{name}", name=f"rowtot_{name}")
        nc.vector.reduce_sum(rowtot[:], abscol[:], axis=AX.X)
        ps = psred.tile([P, 1], dt.float32, tag="psred", bufs=1, name="psred")
        nc.tensor.matmul(ps[:], self.ones[:], rowtot[:], start=True, stop=True)
        wm = self.small.tile([P, 1], dt.float32, tag=f"wm_{name}", name=f"wm_{name}")
        nc.scalar.mul(wm[:], ps[:], 1.0 / (n_oc * P * D))
        return wm

    def load_x(self, x_dram, tagpfx, tagbase="xt2", wait_ms=None, pool=None):
        """Load a [TPC, D] activation as two [P, 2, D] tiles; return the four
        [P, D] per-token-tile APs (token t = tc*128 + p)."""
        xv = x_dram.rearrange("(h tc p) d -> p h tc d", p=P, h=2)
        aps = []
        for h in range(2):
            xt = (pool or self.live).tile([P, 2, D], dt.float32,
                                          tag=f"{tagbase}_{h}",
                                          name=f"{tagpfx}{h}")
            with self.tc.tile_wait_until(wait_ms or 0,
                                         enable=wait_ms is not None):
                self.nc.sync.dma_start(xt[:], xv[:, h])
            aps.extend([xt[:, 0, :], xt[:, 1, :]])
        return aps

    def quant(self, specs, pe_t=False, psp=None, ssq_dve=False, drain="act"):
        """specs: list of (x_aps, GT, name). Quantize several tensors with
        ACT ops grouped by function (fewer activation-table switches).
        Returns dict name -> os [P, NTC] raw out-scale. pe_t: transpose the
        rounded ints on the PE instead of the DMA xbar (psp = psum pool).
        ssq_dve: compute the square-sums on DVE (tensor_tensor_reduce)
        instead of ACT, freeing the ACT queue for the critical path."""
        nc = self.nc
        sm = self.small

        def st(tag, name):
            return sm.tile([P, NTC], dt.float32, tag=f"{tag}_{name}",
                           name=f"{tag}_{name}")

        scr = self.pipe.tile([P, D], dt.bfloat16, tag="scr", bufs=1, name="scr")
        out = {}
        for x_aps, GT, name in specs:
            ssq, amax = st("ssq", name), st("amax", name)
            u, c, amn, os, ra, m1 = (st("u", name), st("c", name),
                                     st("amn", name), st("os", name),
                                     st("ra", name), st("m1", name))
            for tc in range(NTC):
                cs = slice(tc, tc + 1)
                if ssq_dve:
                    nc.vector.tensor_tensor_reduce(
                        scr[:], x_aps[tc], x_aps[tc], 1.0, 0.0,
                        ALU.mult, ALU.add, ssq[:, cs])
                else:
                    nc.scalar.activation(
                        scr[:], x_aps[tc], AF.Square, accum_out=ssq[:, cs])
                nc.vector.tensor_reduce(
                    amax[:, cs], x_aps[tc], AX.X, ALU.max,
                    apply_absolute_value=True,
                )
                nc.scalar.activation(u[:, cs], ssq[:, cs], AF.Sqrt,
                                     bias=self.eps_rms[:])
                nc.vector.reciprocal(c[:, cs], u[:, cs])
                nc.vector.tensor_tensor(amn[:, cs], c[:, cs], amax[:, cs],
                                        ALU.mult)
                nc.vector.tensor_scalar_max(amn[:, cs], amn[:, cs], 1e-5)
                nc.vector.tensor_scalar_mul(os[:, cs], amn[:, cs], 1.0 / 127.0)
                nc.vector.reciprocal(ra[:, cs], amn[:, cs])
                nc.vector.tensor_tensor(m1[:, cs], c[:, cs], ra[:, cs],
                                        ALU.mult)
                nc.vector.tensor_scalar_mul(m1[:, cs], m1[:, cs], 127.0)
                tr = self.pipe.tile([P, D], dt.float32, tag="tr", bufs=2,
                                    name="tr")
                nc.scalar.activation(
                    tr[:], x_aps[tc], AF.Copy, bias=MAGIC, scale=m1[:, cs])
                g = self.pipe.tile([P, D], dt.bfloat16, tag="gtile", bufs=2,
                                   name="g")
                nc.vector.tensor_scalar_sub(g[:], tr[:], MAGIC)
                if pe_t:
                    psg = psp.tile([P, DC, P], dt.bfloat16, tag="gtp",
                                   bufs=1, name="psg")
                    for dc_ in range(DC):
                        nc.tensor.matmul(
                            psg[:, dc_, :], g[:, dc_ * P:(dc_ + 1) * P],
                            self.ident_bf[:], is_transpose=True,
                            start=True, stop=True)
                    if drain == "act":
                        nc.scalar.copy(GT[:, tc], psg[:])
                    else:
                        nc.vector.tensor_copy(GT[:, tc], psg[:])
                else:
                    nc.sync.dma_start_transpose(GT[:, tc], g[:])
            out[name] = os
        return out

    def os_row(self, os_col, name):
        """[P, NTC] per-token column (t = tc*128+p) -> [P, TPC] fp32
        broadcast row via a DRAM bounce."""
        nc = self.nc
        scratch = self.dram.tile([1, TPC], dt.float32, tag=f"osrow_d_{name}",
                                 name=f"osrow_d_{name}")
        nc.gpsimd.dma_start(scratch[0].rearrange("(c p) -> p c", p=P), os_col[:])
        row = self.small.tile([P, TPC], dt.float32, tag=f"osrow_{name}",
                              name=f"osrow_{name}")
        nc.gpsimd.dma_start(row[:], scratch[:].to_broadcast((P, TPC)))
        return row

    def mul_wm(self, os, wm, name, extra=None):
        out = self.small.tile([P, NTC], dt.float32, tag=f"oss_{name}",
                              name=f"oss_{name}")
        self.nc.vector.tensor_tensor(
            out[:], os[:], wm[:, 0:1].to_broadcast((P, NTC)), ALU.mult)
        if extra is not None:
            self.nc.vector.tensor_scalar_mul(out[:], out[:], extra)
        return out


def build_nc(zb: bool, zln: bool):
    """zb: all projection biases zero; zln: ln_g == 1 and ln_b == 0."""
    nc = bass.Bass()
    f32, bf16 = dt.float32, dt.bfloat16

    xq_d = nc.dram_tensor("xq", [TPC, D], f32, kind="ExternalInput")
    xk_d = nc.dram_tensor("xk", [TPC, D], f32, kind="ExternalInput")
    xv_d = nc.dram_tensor("xv", [TPC, D], f32, kind="ExternalInput")
    wq_d = nc.dram_tensor("wq", [D, D], f32, kind="ExternalInput")
    wk_d = nc.dram_tensor("wk", [KVE, D], f32, kind="ExternalInput")
    wv_d = nc.dram_tensor("wv", [KVE, D], f32, kind="ExternalInput")
    wo_d = nc.dram_tensor("wo", [D, D], f32, kind="ExternalInput")
    if not zb:
        bq_d = nc.dram_tensor("bq", [1, D], f32, kind="ExternalInput")
        bk_d = nc.dram_tensor("bk", [1, KVE], f32, kind="ExternalInput")
        bv_d = nc.dram_tensor("bv", [1, KVE], f32, kind="ExternalInput")
        bo_d = nc.dram_tensor("bo", [1, D], f32, kind="ExternalInput")
    if not zln:
        g_d = nc.dram_tensor("g_ln", [1, D], f32, kind="ExternalInput")
        bl_d = nc.dram_tensor("b_ln", [1, D], f32, kind="ExternalInput")
    ident_d = nc.dram_tensor("ident", [P, P], f32, kind="ExternalInput")
    y_d = nc.dram_tensor("y", [TPC, D], f32, kind="ExternalOutput")

    groups = [[0, 1, 2, 3], [4, 5, 6, 7]]

    with tile.TileContext(nc) as tc_, contextlib.ExitStack() as ctx:
        em = _Emit(nc, tc_, ctx)
        small, persist, pipe, dram = em.small, em.persist, em.pipe, em.dram
        nc.sync.dma_start(em.ident[:], ident_d[:, :])
        nc.vector.tensor_copy(em.ident_bf[:], em.ident[:])

        # persistent SBUF structures
        swoT = persist.tile([P, QH, DC, P], bf16, tag="swoT", name="swoT")
        swkT = persist.tile([P, KVH, DC, P], bf16, tag="swkT", name="swkT")
        swvT = persist.tile([P, KVH, DC, P], bf16, tag="swvT", name="swvT")
        GlnT = persist.tile([P, NTC, DC, P], bf16, tag="GlnT", name="GlnT")
        # Q^T: [head_dim partition, head, token]
        qT_sb = persist.tile([P, QH, TPC], bf16, tag="qT_sb", name="qT_sb")
        kT_sb = persist.tile([P, KVH, S], bf16, tag="kT_sb", name="kT_sb")
        # V scaled by its per-source-token out-scale, key-major:
        # [key%128, key chunk, vdim (kv*128+d)]
        v_sc = persist.tile([P, NKC, KVE], bf16, tag="v_sc", name="v_sc")
        x_sb = persist.tile([P, NTC, D], f32, tag="x_sb", name="x_sb")
        # per-key exp scales, gathered with K
        osk_all = persist.tile([P, NKC], f32, tag="osk_all", name="osk_all")
        # incremental layernorm stats, accumulated per attention head-group
        s1p = persist.tile([P, NTC, 4], f32, tag="s1p", name="s1p")
        ssqp = persist.tile([P, NTC, 4], f32, tag="ssqp", name="ssqp")

        mid_cm = tc_.tile_pool(name="mid", bufs=1)
        mid = mid_cm.__enter__()
        em.wpool = mid
        swqT = mid.tile([P, QH, DC, P], bf16, tag="swqT", name="swqT")
        GqT = mid.tile([P, NTC, DC, P], bf16, tag="GqT", name="GqT")
        GkT = mid.tile([P, NTC, DC, P], bf16, tag="GkT", name="GkT")
        GvT = mid.tile([P, NTC, DC, P], bf16, tag="GvT", name="GvT")

        # two pipelined collectives: K (ints + key scales) fires first so
        # scores can start while V is still on the wire.
        #   K buffer: [ k ints (oc p t) | osk (p c) f32 ]
        #   V buffer: [ v ints (tc p o) | osv (p c) f32 ]
        KC_SC = KVE * TPC
        KC_N = KC_SC + 2 * TPC
        cck_in = dram.tile([KC_N], bf16, tag="cck_in", name="cck_in")
        cck_out = dram.tile([GRP, KC_N], bf16, tag="cck_out", name="cck_out")
        VC_SC = TPC * KVE
        ccv_in = dram.tile([VC_SC], bf16, tag="ccv_in", name="ccv_in")
        ccv_out = dram.tile([GRP, VC_SC], bf16, tag="ccv_out", name="ccv_out")

        with tc_.tile_pool(name="ps1", bufs=1, space="PSUM") as ps1:
            # ---------- K path first: load, quantize, project, scatter ------
            # (k weight load leads the DMA queue so sign(w_k) never blocks
            # the ACT queue ahead of the quant rounds)
            wmk = em.weight_prep(wk_d, KVH, swkT, "k", ps1, pe_t=True)
            xk_t = em.load_x(xk_d, "xk")
            xv_t = em.load_x(xv_d, "xv", tagbase="xv2", pool=mid)

            onecol = None
            if not zb:
                onecol = small.tile([P, NTC], f32, tag="onecol", name="onecol")
                nc.vector.memset(onecol[:], 1.0)

            oss_k = em.quant([(xk_t, GkT, "k")], pe_t=True, psp=ps1)
            osk_s = em.mul_wm(oss_k["k"], wmk, "k")
            nc.gpsimd.dma_start(
                cck_in[KC_SC:KC_N].bitcast(f32).rearrange("(p c) -> p c", p=P),
                osk_s[:] if zb else onecol[:])

            kT_loc = mid.tile([P, KVH, TPC], bf16, tag="kT_loc", name="kT_loc")
            v_loc = mid.tile([P, NTC, KVE], bf16, tag="v_loc", name="v_loc")
            if not zb:
                bk_sb = small.tile([P, KVH], f32, tag="bk_sb", name="bk_sb")
                nc.sync.dma_start(bk_sb[:], bk_d[0].rearrange("(c p) -> p c", p=P))
                oskb_row = em.os_row(osk_s, "oskb")
                vb_row = small.tile([P, KVE], f32, tag="vb_row", name="vb_row")
                nc.gpsimd.dma_start(vb_row[:], bv_d[:].to_broadcast((P, KVE)))
            # K: [o, t] orientation
            for oc in range(KVH):
                psum = ps1.tile([P, TPC], f32, tag="proj", bufs=2, name="pj")
                for dc_ in range(DC):
                    nc.tensor.matmul(
                        psum[:], swkT[:, oc, dc_, :], GkT[:, :, dc_, :],
                        start=(dc_ == 0), stop=(dc_ == DC - 1),
                    )
                if zb:
                    nc.vector.tensor_copy(kT_loc[:, oc], psum[:])
                else:
                    tmp = pipe.tile([P, TPC], f32, tag="kvtmp", bufs=2,
                                    name="kvtmp")
                    nc.vector.tensor_tensor(tmp[:], psum[:], oskb_row[:], ALU.mult)
                    nc.vector.tensor_scalar(
                        kT_loc[:, oc], tmp[:], bk_sb[:, oc:oc + 1], None, ALU.add)
            nc.gpsimd.dma_start(
                cck_in[0:KC_SC].rearrange("(oc p t) -> p oc t", p=P, t=TPC),
                kT_loc[:])
            nc.gpsimd.collective_compute(
                "AllGather", ALU.bypass, replica_groups=groups,
                ins=[cck_in.opt()], outs=[cck_out.opt()])

            # ---------- V path (square-sums on DVE to unclog ACT) -----------
            wmv = em.weight_prep(wv_d, KVH, swvT, "v", ps1, pe_t=True)
            oss_v = em.quant([(xv_t, GvT, "v")], pe_t=True, psp=ps1, ssq_dve=True)
            osv_s = em.mul_wm(oss_v["v"], wmv, "v")
            # V: token-major [t, o] orientation, straight into the cc layout
            for tcc in range(NTC):
                psum = ps1.tile([P, KVE], f32, tag="projv", bufs=2, name="pv")
                for dc_ in range(DC):
                    nc.tensor.matmul(
                        psum[:], GvT[:, tcc, dc_, :],
                        swvT[:, :, dc_, :],
                        start=(dc_ == 0), stop=(dc_ == DC - 1),
                    )
                if zb:
                    nc.vector.tensor_scalar(
                        v_loc[:, tcc], psum[:], osv_s[:, tcc:tcc + 1], None,
                        ALU.mult)
                else:
                    tmp2 = pipe.tile([P, KVE], f32, tag="vtmp", bufs=2,
                                     name="vtmp")
                    nc.vector.tensor_scalar(
                        tmp2[:], psum[:], osv_s[:, tcc:tcc + 1], None, ALU.mult)
                    nc.vector.tensor_tensor(
                        v_loc[:, tcc], tmp2[:], vb_row[:], ALU.add)
            nc.gpsimd.dma_start(
                ccv_in[:].rearrange("(tc p o) -> p tc o", p=P, o=KVE),
                v_loc[:])
            nc.gpsimd.collective_compute(
                "AllGather", ALU.bypass, replica_groups=groups,
                ins=[ccv_in.opt()], outs=[ccv_out.opt()])

            # ---------- overlaps collective: q/o weights + Q^T ----------
            # (everything here avoids DmaTranspose, which would serialize
            # against the in-flight collective)
            xq_t = em.load_x(xq_d, "xq", wait_ms=0.034)
            wmq = em.weight_prep(wq_d, QH, swqT, "q", ps1, pe_t=True,
                                 wait_ms=0.036, drain="dve")
            osq = em.quant([(xq_t, GqT, "q")], pe_t=True, psp=ps1,
                           drain="dve")["q"]
            osq_s = em.mul_wm(osq, wmq, "q", extra=INV_SQRT_HD)
            osq_row = em.os_row(osq_s, "osq")
            if not zb:
                qb_sb = small.tile([P, QH], f32, tag="qb_sb", name="qb_sb")
                nc.sync.dma_start(qb_sb[:], bq_d[0].rearrange("(c p) -> p c", p=P))
                # reference scales q (incl. bias) by 1/sqrt(hd)
                nc.vector.tensor_scalar_mul(qb_sb[:], qb_sb[:], INV_SQRT_HD)
            # Q^T directly: per head, out = [head_dim, token]
            for h in range(QH):
                psq = ps1.tile([P, TPC], f32, tag="proj", bufs=2, name="pq")
                for dc_ in range(DC):
                    nc.tensor.matmul(
                        psq[:], swqT[:, h, dc_, :], GqT[:, :, dc_, :],
                        start=(dc_ == 0), stop=(dc_ == DC - 1),
                    )
                if zb:
                    nc.vector.tensor_tensor(
                        qT_sb[:, h, :], psq[:], osq_row[:], ALU.mult)
                else:
                    tmpq = pipe.tile([P, TPC], f32, tag="qtmp", bufs=2,
                                     name="qtmp")
                    nc.vector.tensor_tensor(tmpq[:], psq[:], osq_row[:], ALU.mult)
                    nc.vector.tensor_scalar(
                        qT_sb[:, h, :], tmpq[:], qb_sb[:, h:h + 1], None, ALU.add)

            # ---------- land gathered K/V (K first — scores need it) --------
            nc.sync.dma_start(
                osk_all[:].rearrange("p (s c) -> p s c", s=GRP),
                cck_out[:, KC_SC:KC_N].bitcast(f32)
                .rearrange("s (p c) -> p s c", p=P),
            )
            for oc in range(KVH):
                o0 = oc * P * TPC
                nc.sync.dma_start(
                    kT_sb[:, oc].rearrange("p (s t) -> p s t", t=TPC),
                    cck_out[:, o0:o0 + P * TPC].rearrange("s (p t) -> p s t",
                                                          p=P),
                )
        mid_cm.__exit__(None, None, None)


        # ---------- attention (AV-transposed, software-pipelined) ----------
        # Per head-pair group g: sc(g) = scores+exp (probs in two kc-halves),
        # av(g) = AV^T + denominator matmuls + divide, pst(g) = PE transpose
        # back token-major + Pool drain + LN stats. Emission order
        #   sc0 sc1 av0 sc2 pst0 av1 sc3 pst1 av2 pst2 av3 pst3
        # keeps the PE sequencer free of head-of-line waits: av(g) psum reuse
        # and pst(g)'s divide dependency are always covered by an interleaved
        # scores block.
        with (
            tc_.tile_pool(name="ps2", bufs=1, space="PSUM") as ps2,
            tc_.tile_pool(name="probsp", bufs=1) as probsp,
        ):
            HKC = NKC // 2
            probs_t = {}

            def sc_block(g):
                kv, hp = divmod(g, 2)
                h0 = kv * 4 + hp * 2
                halves = []
                for hh in range(2):
                    ph = probsp.tile([P, HKC, 2, TPC], bf16, tag="probs",
                                     bufs=3, name=f"probs{g}_{hh}")
                    halves.append(ph)
                    for kk in range(HKC):
                        kc = hh * HKC + kk
                        ps_s = ps2.tile([P, 2, TPC], f32, tag="scores",
                                        bufs=2, name="ps_s")
                        nc.tensor.matmul(
                            ps_s[:], kT_sb[:, kv, kc * P:(kc + 1) * P],
                            qT_sb[:, h0:h0 + 2, :], start=True, stop=True,
                        )
                        nc.scalar.activation(
                            ph[:, kk], ps_s[:],
                            AF.Exp, scale=osk_all[:, kc:kc + 1],
                        )
                probs_t[g] = halves

            def av_block(g):
                kv, hp = divmod(g, 2)
                av = ps2.tile([P, 2, TPC], f32, tag="av", bufs=1, name="av")
                dn = ps2.tile([P, 2, TPC], f32, tag="denom", bufs=1, name="dn")
                for kc in range(NKC):
                    ph = probs_t[g][kc // HKC]
                    kk = kc % HKC
                    for j in range(2):
                        nc.tensor.matmul(
                            av[:, j, :], v_sc[:, kc, kv * P:(kv + 1) * P],
                            ph[:, kk, j, :],
                            start=(kc == 0), stop=(kc == NKC - 1),
                        )
                    nc.tensor.matmul(
                        dn[:], em.ones_bf[:], ph[:, kk],
                        start=(kc == 0), stop=(kc == NKC - 1),
                    )
                rden_p = probsp.tile([P, 2, TPC], f32, tag="rden", bufs=2,
                                     name="rden")
                nc.vector.reciprocal(rden_p[:], dn[:])
                xT_p = probsp.tile([P, 2, TPC], f32, tag="xT", bufs=2,
                                   name="xT")
                for j in range(2):
                    nc.vector.tensor_tensor(
                        xT_p[:, j, :], av[:, j, :], rden_p[:, j, :], ALU.mult)
                return xT_p

            def pst_block(g, xT_p):
                kv, hp = divmod(g, 2)
                h0 = kv * 4 + hp * 2
                pst = ps2.tile([P, 2, TPC], f32, tag="scores", bufs=2,
                               name="pst")
                for j in range(2):
                    h = h0 + j
                    for tcc in range(NTC):
                        nc.tensor.matmul(
                            pst[:, j, tcc * P:(tcc + 1) * P],
                            xT_p[:, j, tcc * P:(tcc + 1) * P],
                            em.ident[:],
                            is_transpose=True, start=True, stop=True,
                        )
                        nc.vector.tensor_copy(
                            x_sb[:, tcc, h * P:(h + 1) * P],
                            pst[:, j, tcc * P:(tcc + 1) * P])
                scrA = pipe.tile([P, 2 * P], f32, tag="scrA", bufs=1,
                                 name="scrA")
                for tcc in range(NTC):
                    xg = x_sb[:, tcc, h0 * P:(h0 + 2) * P]
                    nc.vector.reduce_sum(
                        s1p[:, tcc, g:g + 1], xg, axis=AX.X)
                    nc.vector.tensor_tensor_reduce(
                        scrA[:], xg, xg, 1.0, 0.0, ALU.mult, ALU.add,
                        ssqp[:, tcc, g:g + 1])

            sc_block(0)
            sc_block(1)
            # land gathered V here: the wait hint keeps the scheduler's
            # internal sim from ordering the scores behind the V collective
            # (it would then compress score waits onto the V DMA queue).
            # V was pre-scaled by its out-scale on the source core.
            with tc_.tile_wait_until(0.125):
                for s_ in range(GRP):
                    nc.sync.dma_start(
                        v_sc[:, s_ * NTC:(s_ + 1) * NTC, :],
                        ccv_out[s_].rearrange("(tc p o) -> p tc o",
                                              p=P, o=KVE),
                    )
            xt0 = av_block(0)
            sc_block(2)
            pst_block(0, xt0)
            xt1 = av_block(1)
            sc_block(3)
            pst_block(1, xt1)
            xt2_ = av_block(2)
            pst_block(2, xt2_)
            xt3 = av_block(3)
            pst_block(3, xt3)

        # ---------- layernorm + final quant + output projection ----------
        with (
            tc_.tile_pool(name="ps3", bufs=1, space="PSUM") as ps3,
            tc_.tile_pool(name="opool", bufs=1) as opool,
        ):
            wmo = em.weight_prep(wo_d, QH, swoT, "o", ps3, wpool=opool,
                                 pe_t=True, drain="dve")
            sm = small

            def st(tag):
                return sm.tile([P, NTC], dt.float32, tag=tag, name=tag)

            s1, ssql = st("s1_ln"), st("ssq_ln")
            nc.vector.reduce_sum(s1[:], s1p[:], axis=AX.X)
            nc.vector.reduce_sum(ssql[:], ssqp[:], axis=AX.X)
            mu, e2, m2, var, sd, rstd, nmu = (
                st("mu"), st("e2"), st("m2"), st("var"), st("sd"), st("rstd"),
                st("nmu"))
            ssq2, amax2 = st("ssq2"), st("amax2")
            u2, c2, amn2, osl, ra2, m1l = (
                st("u2"), st("c2"), st("amn2"), st("osl"), st("ra2"), st("m1l"))
            syl = st("syl")
            scr2 = pipe.tile([P, D], dt.bfloat16, tag="scr", bufs=1, name="scr2")

            if not zln:
                g_row = persist.tile([P, D], dt.float32, tag="g_row", name="g_row")
                nc.gpsimd.dma_start(g_row[:], g_d[:].to_broadcast((P, D)))
                b_row = persist.tile([P, D], dt.float32, tag="b_row", name="b_row")
                nc.gpsimd.dma_start(b_row[:], bl_d[:].to_broadcast((P, D)))
            if not zb:
                ob_row = persist.tile([P, D], dt.float32, tag="ob_row",
                                      name="ob_row")
                nc.gpsimd.dma_start(ob_row[:], bo_d[:].to_broadcast((P, D)))

            yv = y_d.rearrange("(tc p) o -> p tc o", p=P)
            # two-token-tile halves: out-proj of half 0 overlaps the ln/quant
            # chain of half 1
            for hf in range(2):
                hs = slice(2 * hf, 2 * hf + 2)
                nc.vector.tensor_scalar_mul(mu[:, hs], s1[:, hs], 1.0 / D)
                nc.vector.tensor_scalar_mul(e2[:, hs], ssql[:, hs], 1.0 / D)
                nc.vector.tensor_tensor(m2[:, hs], mu[:, hs], mu[:, hs], ALU.mult)
                nc.vector.tensor_tensor(var[:, hs], e2[:, hs], m2[:, hs],
                                        ALU.subtract)
                nc.scalar.activation(sd[:, hs], var[:, hs], AF.Sqrt,
                                     bias=em.eps_ln[:])
                nc.vector.reciprocal(rstd[:, hs], sd[:, hs])
                nc.vector.tensor_scalar_mul(nmu[:, hs], mu[:, hs], -1.0)

                lt2 = em.live.tile([P, 2, D], dt.float32, tag=f"xt2_{hf}",
                                   name=f"lt2_{hf}")
                lt_aps = []
                for i in range(2):
                    tcc = 2 * hf + i
                    nc.vector.tensor_scalar(
                        lt2[:, i, :], x_sb[:, tcc], nmu[:, tcc:tcc + 1],
                        rstd[:, tcc:tcc + 1], ALU.add, ALU.mult,
                    )
                    if not zln:
                        nc.vector.tensor_tensor(
                            lt2[:, i, :], lt2[:, i, :], g_row[:], ALU.mult)
                        nc.vector.tensor_tensor(
                            lt2[:, i, :], lt2[:, i, :], b_row[:], ALU.add)
                    lt_aps.append(lt2[:, i, :])

                # bitlinear quant of this half
                for i, lt in enumerate(lt_aps):
                    tcc = 2 * hf + i
                    nc.scalar.activation(
                        scr2[:], lt, AF.Square, accum_out=ssq2[:, tcc:tcc + 1])
                    nc.vector.tensor_reduce(
                        amax2[:, tcc:tcc + 1], lt, AX.X, ALU.max,
                        apply_absolute_value=True)
                nc.scalar.activation(u2[:, hs], ssq2[:, hs], AF.Sqrt,
                                     bias=em.eps_rms[:])
                nc.vector.reciprocal(c2[:, hs], u2[:, hs])
                nc.vector.tensor_tensor(amn2[:, hs], c2[:, hs], amax2[:, hs],
                                        ALU.mult)
                nc.vector.tensor_scalar_max(amn2[:, hs], amn2[:, hs], 1e-5)
                nc.vector.tensor_scalar_mul(osl[:, hs], amn2[:, hs], 1.0 / 127.0)
                nc.vector.reciprocal(ra2[:, hs], amn2[:, hs])
                nc.vector.tensor_tensor(m1l[:, hs], c2[:, hs], ra2[:, hs],
                                        ALU.mult)
                nc.vector.tensor_scalar_mul(m1l[:, hs], m1l[:, hs], 127.0)
                nc.vector.tensor_tensor(
                    syl[:, hs], osl[:, hs],
                    wmo[:, 0:1].to_broadcast((P, 2)), ALU.mult)
                for i, lt in enumerate(lt_aps):
                    tcc = 2 * hf + i
                    tr = pipe.tile([P, D], dt.float32, tag="tr", bufs=2,
                                   name="tr")
                    nc.scalar.activation(
                        tr[:], lt, AF.Copy, bias=MAGIC,
                        scale=m1l[:, tcc:tcc + 1])
                    g = pipe.tile([P, D], dt.bfloat16, tag="gtile", bufs=2,
                                  name="g")
                    nc.vector.tensor_scalar_sub(g[:], tr[:], MAGIC)
                    nc.sync.dma_start_transpose(GlnT[:, tcc], g[:])
                for i in range(2):
                    tcc = 2 * hf + i
                    for og in range(2):
                        psum = ps3.tile([P, TPC], dt.float32, tag="yproj",
                                        bufs=3, name="py")
                        for dc_ in range(DC):
                            nc.tensor.matmul(
                                psum[:], GlnT[:, tcc, dc_, :],
                                swoT[:, og * 4:(og + 1) * 4, dc_, :],
                                start=(dc_ == 0), stop=(dc_ == DC - 1),
                            )
                        yt = pipe.tile([P, TPC], dt.float32, tag="yt", bufs=2,
                                       name="yt")
                        nc.vector.tensor_scalar(
                            yt[:], psum[:], syl[:, tcc:tcc + 1], None, ALU.mult)
                        if not zb:
                            nc.vector.tensor_tensor(
                                yt[:], yt[:],
                                ob_row[:, og * TPC:(og + 1) * TPC], ALU.add)
                        nc.sync.dma_start(
                            yv[:, tcc, og * TPC:(og + 1) * TPC], yt[:])

    _split_multiwaits(nc)
    return nc


def kernel(**inputs):
    from concourse.bass_utils import run_bass_kernel_spmd

    def arr(name):
        return np.ascontiguousarray(np.asarray(inputs[name], dtype=np.float32))

    q, k, v = arr("query"), arr("key"), arr("value")
    qw, kw, vw, ow = arr("q_w"), arr("k_w"), arr("v_w"), arr("out_w")
    qb, kb, vb, ob = arr("q_b"), arr("k_b"), arr("v_b"), arr("out_b")
    lg, lb = arr("ln_g"), arr("ln_b")

    zb = not (qb.any() or kb.any() or vb.any() or ob.any())
    zln = bool(np.all(lg == 1.0)) and not lb.any()

    key = (zb, zln)
    if key not in _BUILT:
        _BUILT[key] = build_nc(zb, zln)
    nc = _BUILT[key]

    qf = q.reshape(B * S, D)
    kf = k.reshape(B * S, D)
    vf = v.reshape(B * S, D)
    in_maps = []
    for c in range(N_CORES):
        sl = slice(c * TPC, (c + 1) * TPC)
        m = {
            "xq": qf[sl], "xk": kf[sl], "xv": vf[sl],
            "wq": qw, "wk": kw, "wv": vw, "wo": ow,
            "ident": np.eye(P, dtype=np.float32),
        }
        if not zb:
            m["bq"] = qb.reshape(1, D)
            m["bk"] = kb.reshape(1, KVE)
            m["bv"] = vb.reshape(1, KVE)
            m["bo"] = ob.reshape(1, D)
        if not zln:
            m["g_ln"] = lg.reshape(1, D)
            m["b_ln"] = lb.reshape(1, D)
        in_maps.append(m)

    res = run_bass_kernel_spmd(nc, in_maps, core_ids=list(range(N_CORES)))
    y = np.concatenate([res.results[c]["y"] for c in range(N_CORES)], axis=0)
    return y.reshape(B, S, D).astype(np.float32)


# revision 62
# speedup vs baseline: 1.0152x; 1.0152x over previous
"""BitMGQA forward for Trainium2, 8-core SPMD Bass/Tile kernel.

Sharding: the B*S = 4096 tokens are split into 8 slices of 512 (cores 0-3 =
batch 0, cores 4-7 = batch 1). Each core quantizes + projects its own token
slice for Q/K/V; the per-batch K/V (plus their per-token output scales) are
AllGathered across each 4-core batch group in one merged collective;
attention, layernorm and the output projection are then fully local to each
core's 512 query tokens.

BitLinear structure is exploited: activation quantization produces integers in
[-128, 127] and weight binarization produces +-1, both exactly representable
in bf16, so every projection matmul runs on the PE in bf16 with fp32 PSUM
accumulation exactly; rmsnorm / quant / weight scales fold into one per-token
fp32 scale applied to the matmul output (or, for K, into the softmax exp's
per-key scale operand).

Attention runs "AV-transposed": scores stay key-major ([key, token] psum,
exp'd in place with the per-key scale), and AV contracts keys with V as the
128-wide stationary so the moving dim is 512 tokens (big frees, few
instructions). Softmax denominators come from an all-ones stationary matmul
over the same probs. The [head_dim, token] attention output is divided by the
denominator, transposed back token-major on the PE (fp32-exact) with Pool
draining PSUM, and the layernorm/quant/out-proj tail runs token-major.

Scheduling: only the K/V path runs before the AllGather; Q quant/projection,
q/o weight prep and the *local* quarter of scores+exp fill the collective
window.
"""
import contextlib

import numpy as np

import concourse.bass as bass
import concourse.mybir as mybir
import concourse.tile as tile

dt = mybir.dt
AF = mybir.ActivationFunctionType
ALU = mybir.AluOpType
AX = mybir.AxisListType

# problem dims (hardcoded per contract)
B, S, D = 2, 2048, 1024
N_CORES = 8
GRP = 4                    # cores per batch group
TPC = (B * S) // N_CORES   # 512 tokens per core
P = 128
NTC = TPC // P             # 4 token tiles per core
DC = D // P                # 8 contraction chunks
KVE = 256
QH, KVH, HD = 8, 2, 128
NKC = S // P               # 16 key chunks per batch
LKC = TPC // P             # 4 key chunks produced locally
MAGIC = 12582912.0         # 1.5 * 2**23: (x + MAGIC) - MAGIC == rint(x)
INV_SQRT_HD = float(HD) ** -0.5
LN_EPS = 1e-5
RMS_EPS = 1e-6

_BUILT = {}


def _split_multiwaits(nc, max_waits=1):
    """The pinned walrus rejects >1 sync-wait per instruction ("Too many sync
    wait commands"). Split extras onto single-wait NoOps inserted before the
    offending instruction on the same engine (sequencer stalls in order, so
    semantics are identical)."""
    for f in nc.m.functions:
        for bb in f.blocks:
            insts = list(bb.instructions)
            if not any(
                i.sync_info is not None and len(i.sync_info.on_wait) > max_waits
                for i in insts
            ):
                continue
            new_insts = []
            for ins in insts:
                si = ins.sync_info
                if si is not None and len(si.on_wait) > max_waits:
                    waits = list(si.on_wait)
                    for k, w in enumerate(waits[:-max_waits]):
                        new_insts.append(mybir.InstNoOp(
                            name=f"{ins.name}-wsplit{k}",
                            engine=ins.engine,
                            sync_info=mybir.SyncInfo(on_wait=[w], on_update=[]),
                        ))
                    ins.sync_info = mybir.SyncInfo(
                        on_wait=waits[-max_waits:], on_update=list(si.on_update)
                    )
                new_insts.append(ins)
            bb.instructions = new_insts


class _Emit:
    """Per-build emission state."""

    def __init__(self, nc, tc_, ctx):
        self.nc = nc
        self.tc = tc_
        f32 = dt.float32
        self.small = ctx.enter_context(tc_.tile_pool(name="small", bufs=1))
        self.persist = ctx.enter_context(tc_.tile_pool(name="persist", bufs=1))
        self.pipe = ctx.enter_context(tc_.tile_pool(name="pipe", bufs=3))
        self.live = ctx.enter_context(tc_.tile_pool(name="live", bufs=1))
        self.dram = ctx.enter_context(
            tc_.tile_pool(name="dram", bufs=1, space="DRAM"))
        self.ones = self.small.tile([P, P], f32, tag="ones128", name="ones")
        nc.vector.memset(self.ones[:], 1.0)
        self.ones_bf = self.small.tile([P, P], dt.bfloat16, tag="ones128b",
                                       name="ones_bf")
        nc.vector.memset(self.ones_bf[:], 1.0)
        self.eps_rms = self.small.tile([P, 1], f32, tag="eps_rms", name="eps_rms")
        nc.vector.memset(self.eps_rms[:], float(D * RMS_EPS))
        self.eps_ln = self.small.tile([P, 1], f32, tag="eps_ln", name="eps_ln")
        nc.vector.memset(self.eps_ln[:], LN_EPS)
        # identity for PE transposes, loaded from an external input (the
        # gpsimd affine_select path is rejected by this walrus build);
        # build_nc fills these right after construction
        self.ident = self.small.tile([P, P], f32, tag="ident", name="ident")
        self.ident_bf = self.small.tile([P, P], dt.bfloat16, tag="ident_bf",
                                        name="ident_bf")

    # ---- helpers -------------------------------------------------------
    def weight_prep(self, w_dram, n_oc, swT, name, psred, wpool=None,
                    load_eng=None, pe_t=False, wait_ms=None, drain="act"):
        """sign(w)^T into swT [P, n_oc, DC, P] bf16; returns mean|w| as a
        [P, 1] fp32 column replicated across partitions. Loads weight rows
        two 128-row chunks at a time to halve DMA count. pe_t: transpose on
        the PE (Pool drains PSUM) instead of the DMA xbar — required after
        the collective is issued, since DmaTranspose serializes with it."""
        nc = self.nc
        wpool = wpool or self.wpool
        npair = n_oc // 2
        abscol = self.small.tile(
            [P, npair], dt.float32, tag=f"abscol_{name}", name=f"abscol_{name}")
        wv = w_dram.rearrange("(oc p) d -> p oc d", p=P)
        for pr in range(npair):
            wt = wpool.tile([P, 2, D], dt.float32, tag="wtile", bufs=3,
                            name="wt")
            with self.tc.tile_wait_until(wait_ms or 0, enable=wait_ms is not None):
                (load_eng or nc.sync).dma_start(wt[:], wv[:, 2 * pr:2 * pr + 2])
            sw = wpool.tile([P, 2, D], dt.bfloat16, tag="swtile", bufs=2,
                            name="sw")
            nc.scalar.sign(sw[:], wt[:])
            nc.vector.tensor_reduce(
                abscol[:, pr:pr + 1], wt[:], AX.XY, ALU.add,
                apply_absolute_value=True,
            )
            if pe_t:
                pswt = psred.tile([P, 2, DC, P], dt.bfloat16, tag="wtp",
                                  bufs=1, name="pswt")
                for i in range(2):
                    for dc_ in range(DC):
                        nc.tensor.matmul(
                            pswt[:, i, dc_, :], sw[:, i, dc_ * P:(dc_ + 1) * P],
                            self.ident_bf[:], is_transpose=True,
                            start=True, stop=True)
                    if drain == "act":
                        nc.scalar.copy(swT[:, 2 * pr + i], pswt[:, i])
                    else:
                        nc.vector.tensor_copy(swT[:, 2 * pr + i], pswt[:, i])
            else:
                for i in range(2):
                    nc.sync.dma_start_transpose(swT[:, 2 * pr + i], sw[:, i])
        rowtot = self.small.tile(
            [P, 1], dt.float32, tag=f"rowtot_{name}", name=f"rowtot_{name}")
        nc.vector.reduce_sum(rowtot[:], abscol[:], axis=AX.X)
        ps = psred.tile([P, 1], dt.float32, tag="psred", bufs=1, name="psred")
        nc.tensor.matmul(ps[:], self.ones[:], rowtot[:], start=True, stop=True)
        wm = self.small.tile([P, 1], dt.float32, tag=f"wm_{name}", name=f"wm_{name}")
        nc.scalar.mul(wm[:], ps[:], 1.0 / (n_oc * P * D))
        return wm

    def load_x(self, x_dram, tagpfx, tagbase="xt2", wait_ms=None, pool=None):
        """Load a [TPC, D] activation as two [P, 2, D] tiles; return the four
        [P, D] per-token-tile APs (token t = tc*128 + p)."""
        xv = x_dram.rearrange("(h tc p) d -> p h tc d", p=P, h=2)
        aps = []
        for h in range(2):
            xt = (pool or self.live).tile([P, 2, D], dt.float32,
                                          tag=f"{tagbase}_{h}",
                                          name=f"{tagpfx}{h}")
            with self.tc.tile_wait_until(wait_ms or 0,
                                         enable=wait_ms is not None):
                self.nc.sync.dma_start(xt[:], xv[:, h])
            aps.extend([xt[:, 0, :], xt[:, 1, :]])
        return aps

    def quant(self, specs, pe_t=False, psp=None, ssq_dve=False, drain="act"):
        """specs: list of (x_aps, GT, name). Quantize several tensors with
        ACT ops grouped by function (fewer activation-table switches).
        Returns dict name -> os [P, NTC] raw out-scale. pe_t: transpose the
        rounded ints on the PE instead of the DMA xbar (psp = psum pool).
        ssq_dve: compute the square-sums on DVE (tensor_tensor_reduce)
        instead of ACT, freeing the ACT queue for the critical path."""
        nc = self.nc
        sm = self.small

        def st(tag, name):
            return sm.tile([P, NTC], dt.float32, tag=f"{tag}_{name}",
                           name=f"{tag}_{name}")

        scr = self.pipe.tile([P, D], dt.bfloat16, tag="scr", bufs=1, name="scr")
        out = {}
        for x_aps, GT, name in specs:
            ssq, amax = st("ssq", name), st("amax", name)
            u, c, amn, os, ra, m1 = (st("u", name), st("c", name),
                                     st("amn", name), st("os", name),
                                     st("ra", name), st("m1", name))
            for tc in range(NTC):
                cs = slice(tc, tc + 1)
                if ssq_dve:
                    nc.vector.tensor_tensor_reduce(
                        scr[:], x_aps[tc], x_aps[tc], 1.0, 0.0,
                        ALU.mult, ALU.add, ssq[:, cs])
                else:
                    nc.scalar.activation(
                        scr[:], x_aps[tc], AF.Square, accum_out=ssq[:, cs])
                nc.vector.tensor_reduce(
                    amax[:, cs], x_aps[tc], AX.X, ALU.max,
                    apply_absolute_value=True,
                )
                nc.scalar.activation(u[:, cs], ssq[:, cs], AF.Sqrt,
                                     bias=self.eps_rms[:])
                nc.vector.reciprocal(c[:, cs], u[:, cs])
                nc.vector.tensor_tensor(amn[:, cs], c[:, cs], amax[:, cs],
                                        ALU.mult)
                nc.vector.tensor_scalar_max(amn[:, cs], amn[:, cs], 1e-5)
                nc.vector.tensor_scalar_mul(os[:, cs], amn[:, cs], 1.0 / 127.0)
                nc.vector.reciprocal(ra[:, cs], amn[:, cs])
                nc.vector.tensor_tensor(m1[:, cs], c[:, cs], ra[:, cs],
                                        ALU.mult)
                nc.vector.tensor_scalar_mul(m1[:, cs], m1[:, cs], 127.0)
                tr = self.pipe.tile([P, D], dt.float32, tag="tr", bufs=2,
                                    name="tr")
                nc.scalar.activation(
                    tr[:], x_aps[tc], AF.Copy, bias=MAGIC, scale=m1[:, cs])
                g = self.pipe.tile([P, D], dt.bfloat16, tag="gtile", bufs=2,
                                   name="g")
                nc.vector.tensor_scalar_sub(g[:], tr[:], MAGIC)
                if pe_t:
                    psg = psp.tile([P, DC, P], dt.bfloat16, tag="gtp",
                                   bufs=1, name="psg")
                    for dc_ in range(DC):
                        nc.tensor.matmul(
                            psg[:, dc_, :], g[:, dc_ * P:(dc_ + 1) * P],
                            self.ident_bf[:], is_transpose=True,
                            start=True, stop=True)
                    if drain == "act":
                        nc.scalar.copy(GT[:, tc], psg[:])
                    else:
                        nc.vector.tensor_copy(GT[:, tc], psg[:])
                else:
                    nc.sync.dma_start_transpose(GT[:, tc], g[:])
            out[name] = os
        return out

    def os_row(self, os_col, name):
        """[P, NTC] per-token column (t = tc*128+p) -> [P, TPC] fp32
        broadcast row via a DRAM bounce."""
        nc = self.nc
        scratch = self.dram.tile([1, TPC], dt.float32, tag=f"osrow_d_{name}",
                                 name=f"osrow_d_{name}")
        nc.gpsimd.dma_start(scratch[0].rearrange("(c p) -> p c", p=P), os_col[:])
        row = self.small.tile([P, TPC], dt.float32, tag=f"osrow_{name}",
                              name=f"osrow_{name}")
        nc.gpsimd.dma_start(row[:], scratch[:].to_broadcast((P, TPC)))
        return row

    def mul_wm(self, os, wm, name, extra=None):
        out = self.small.tile([P, NTC], dt.float32, tag=f"oss_{name}",
                              name=f"oss_{name}")
        self.nc.vector.tensor_tensor(
            out[:], os[:], wm[:, 0:1].to_broadcast((P, NTC)), ALU.mult)
        if extra is not None:
            self.nc.vector.tensor_scalar_mul(out[:], out[:], extra)
        return out


def build_nc(zb: bool, zln: bool):
    """zb: all projection biases zero; zln: ln_g == 1 and ln_b == 0."""
    nc = bass.Bass()
    f32, bf16 = dt.float32, dt.bfloat16

    xq_d = nc.dram_tensor("xq", [TPC, D], f32, kind="ExternalInput")
    xk_d = nc.dram_tensor("xk", [TPC, D], f32, kind="ExternalInput")
    xv_d = nc.dram_tensor("xv", [TPC, D], f32, kind="ExternalInput")
    wq_d = nc.dram_tensor("wq", [D, D], f32, kind="ExternalInput")
    wk_d = nc.dram_tensor("wk", [KVE, D], f32, kind="ExternalInput")
    wv_d = nc.dram_tensor("wv", [KVE, D], f32, kind="ExternalInput")
    wo_d = nc.dram_tensor("wo", [D, D], f32, kind="ExternalInput")
    if not zb:
        bq_d = nc.dram_tensor("bq", [1, D], f32, kind="ExternalInput")
        bk_d = nc.dram_tensor("bk", [1, KVE], f32, kind="ExternalInput")
        bv_d = nc.dram_tensor("bv", [1, KVE], f32, kind="ExternalInput")
        bo_d = nc.dram_tensor("bo", [1, D], f32, kind="ExternalInput")
    if not zln:
        g_d = nc.dram_tensor("g_ln", [1, D], f32, kind="ExternalInput")
        bl_d = nc.dram_tensor("b_ln", [1, D], f32, kind="ExternalInput")
    ident_d = nc.dram_tensor("ident", [P, P], f32, kind="ExternalInput")
    y_d = nc.dram_tensor("y", [TPC, D], f32, kind="ExternalOutput")

    groups = [[0, 1, 2, 3], [4, 5, 6, 7]]

    with tile.TileContext(nc) as tc_, contextlib.ExitStack() as ctx:
        em = _Emit(nc, tc_, ctx)
        small, persist, pipe, dram = em.small, em.persist, em.pipe, em.dram
        nc.sync.dma_start(em.ident[:], ident_d[:, :])
        nc.vector.tensor_copy(em.ident_bf[:], em.ident[:])

        # persistent SBUF structures
        swoT = persist.tile([P, QH, DC, P], bf16, tag="swoT", name="swoT")
        swkT = persist.tile([P, KVH, DC, P], bf16, tag="swkT", name="swkT")
        swvT = persist.tile([P, KVH, DC, P], bf16, tag="swvT", name="swvT")
        GlnT = persist.tile([P, NTC, DC, P], bf16, tag="GlnT", name="GlnT")
        # Q^T: [head_dim partition, head, token]
        qT_sb = persist.tile([P, QH, TPC], bf16, tag="qT_sb", name="qT_sb")
        kT_sb = persist.tile([P, KVH, S], bf16, tag="kT_sb", name="kT_sb")
        # V scaled by its per-source-token out-scale, key-major:
        # [key%128, key chunk, vdim (kv*128+d)]
        v_sc = persist.tile([P, NKC, KVE], bf16, tag="v_sc", name="v_sc")
        x_sb = persist.tile([P, NTC, D], f32, tag="x_sb", name="x_sb")
        # per-key exp scales, gathered with K
        osk_all = persist.tile([P, NKC], f32, tag="osk_all", name="osk_all")
        # incremental layernorm stats, accumulated per attention head-group
        s1p = persist.tile([P, NTC, 4], f32, tag="s1p", name="s1p")
        ssqp = persist.tile([P, NTC, 4], f32, tag="ssqp", name="ssqp")

        mid_cm = tc_.tile_pool(name="mid", bufs=1)
        mid = mid_cm.__enter__()
        em.wpool = mid
        swqT = mid.tile([P, QH, DC, P], bf16, tag="swqT", name="swqT")
        GqT = mid.tile([P, NTC, DC, P], bf16, tag="GqT", name="GqT")
        GkT = mid.tile([P, NTC, DC, P], bf16, tag="GkT", name="GkT")
        GvT = mid.tile([P, NTC, DC, P], bf16, tag="GvT", name="GvT")

        # two pipelined collectives: K (ints + key scales) fires first so
        # scores can start while V is still on the wire.
        #   K buffer: [ k ints (oc p t) | osk (p c) f32 ]
        #   V buffer: [ v ints (tc p o) | osv (p c) f32 ]
        KC_SC = KVE * TPC
        KC_N = KC_SC + 2 * TPC
        cck_in = dram.tile([KC_N], bf16, tag="cck_in", name="cck_in")
        cck_out = dram.tile([GRP, KC_N], bf16, tag="cck_out", name="cck_out")
        VC_SC = TPC * KVE
        ccv_in = dram.tile([VC_SC], bf16, tag="ccv_in", name="ccv_in")
        ccv_out = dram.tile([GRP, VC_SC], bf16, tag="ccv_out", name="ccv_out")

        with tc_.tile_pool(name="ps1", bufs=1, space="PSUM") as ps1:
            # ---------- K path first: load, quantize, project, scatter ------
            # (k weight load leads the DMA queue so sign(w_k) never blocks
            # the ACT queue ahead of the quant rounds)
            wmk = em.weight_prep(wk_d, KVH, swkT, "k", ps1, pe_t=True)
            xk_t = em.load_x(xk_d, "xk")
            xv_t = em.load_x(xv_d, "xv", tagbase="xv2", pool=mid)

            onecol = None
            if not zb:
                onecol = small.tile([P, NTC], f32, tag="onecol", name="onecol")
                nc.vector.memset(onecol[:], 1.0)

            oss_k = em.quant([(xk_t, GkT, "k")], pe_t=True, psp=ps1)
            osk_s = em.mul_wm(oss_k["k"], wmk, "k")
            nc.gpsimd.dma_start(
                cck_in[KC_SC:KC_N].bitcast(f32).rearrange("(p c) -> p c", p=P),
                osk_s[:] if zb else onecol[:])

            kT_loc = mid.tile([P, KVH, TPC], bf16, tag="kT_loc", name="kT_loc")
            v_loc = mid.tile([P, NTC, KVE], bf16, tag="v_loc", name="v_loc")
            if not zb:
                bk_sb = small.tile([P, KVH], f32, tag="bk_sb", name="bk_sb")
                nc.sync.dma_start(bk_sb[:], bk_d[0].rearrange("(c p) -> p c", p=P))
                oskb_row = em.os_row(osk_s, "oskb")
                vb_row = small.tile([P, KVE], f32, tag="vb_row", name="vb_row")
                nc.gpsimd.dma_start(vb_row[:], bv_d[:].to_broadcast((P, KVE)))
            # K: [o, t] orientation
            for oc in range(KVH):
                psum = ps1.tile([P, TPC], f32, tag="proj", bufs=2, name="pj")
                for dc_ in range(DC):
                    nc.tensor.matmul(
                        psum[:], swkT[:, oc, dc_, :], GkT[:, :, dc_, :],
                        start=(dc_ == 0), stop=(dc_ == DC - 1),
                    )
                if zb:
                    nc.vector.tensor_copy(kT_loc[:, oc], psum[:])
                else:
                    tmp = pipe.tile([P, TPC], f32, tag="kvtmp", bufs=2,
                                    name="kvtmp")
                    nc.vector.tensor_tensor(tmp[:], psum[:], oskb_row[:], ALU.mult)
                    nc.vector.tensor_scalar(
                        kT_loc[:, oc], tmp[:], bk_sb[:, oc:oc + 1], None, ALU.add)
            nc.gpsimd.dma_start(
                cck_in[0:KC_SC].rearrange("(oc p t) -> p oc t", p=P, t=TPC),
                kT_loc[:])
            nc.gpsimd.collective_compute(
                "AllGather", ALU.bypass, replica_groups=groups,
                ins=[cck_in.opt()], outs=[cck_out.opt()])

            # ---------- V path (square-sums on DVE to unclog ACT) -----------
            wmv = em.weight_prep(wv_d, KVH, swvT, "v", ps1, pe_t=True)
            oss_v = em.quant([(xv_t, GvT, "v")], pe_t=True, psp=ps1)
            osv_s = em.mul_wm(oss_v["v"], wmv, "v")
            # V: token-major [t, o] orientation, straight into the cc layout
            for tcc in range(NTC):
                psum = ps1.tile([P, KVE], f32, tag="projv", bufs=2, name="pv")
                for dc_ in range(DC):
                    nc.tensor.matmul(
                        psum[:], GvT[:, tcc, dc_, :],
                        swvT[:, :, dc_, :],
                        start=(dc_ == 0), stop=(dc_ == DC - 1),
                    )
                if zb:
                    nc.vector.tensor_scalar(
                        v_loc[:, tcc], psum[:], osv_s[:, tcc:tcc + 1], None,
                        ALU.mult)
                else:
                    tmp2 = pipe.tile([P, KVE], f32, tag="vtmp", bufs=2,
                                     name="vtmp")
                    nc.vector.tensor_scalar(
                        tmp2[:], psum[:], osv_s[:, tcc:tcc + 1], None, ALU.mult)
                    nc.vector.tensor_tensor(
                        v_loc[:, tcc], tmp2[:], vb_row[:], ALU.add)
            nc.gpsimd.dma_start(
                ccv_in[:].rearrange("(tc p o) -> p tc o", p=P, o=KVE),
                v_loc[:])
            nc.gpsimd.collective_compute(
                "AllGather", ALU.bypass, replica_groups=groups,
                ins=[ccv_in.opt()], outs=[ccv_out.opt()])

            # ---------- overlaps collective: q/o weights + Q^T ----------
            # (everything here avoids DmaTranspose, which would serialize
            # against the in-flight collective)
            xq_t = em.load_x(xq_d, "xq", wait_ms=0.034)
            wmq = em.weight_prep(wq_d, QH, swqT, "q", ps1, pe_t=True,
                                 wait_ms=0.036, drain="dve")
            osq = em.quant([(xq_t, GqT, "q")], pe_t=True, psp=ps1,
                           drain="dve")["q"]
            osq_s = em.mul_wm(osq, wmq, "q", extra=INV_SQRT_HD)
            osq_row = em.os_row(osq_s, "osq")
            if not zb:
                qb_sb = small.tile([P, QH], f32, tag="qb_sb", name="qb_sb")
                nc.sync.dma_start(qb_sb[:], bq_d[0].rearrange("(c p) -> p c", p=P))
                # reference scales q (incl. bias) by 1/sqrt(hd)
                nc.vector.tensor_scalar_mul(qb_sb[:], qb_sb[:], INV_SQRT_HD)
            # Q^T directly: per head, out = [head_dim, token]
            for h in range(QH):
                psq = ps1.tile([P, TPC], f32, tag="proj", bufs=2, name="pq")
                for dc_ in range(DC):
                    nc.tensor.matmul(
                        psq[:], swqT[:, h, dc_, :], GqT[:, :, dc_, :],
                        start=(dc_ == 0), stop=(dc_ == DC - 1),
                    )
                if zb:
                    nc.vector.tensor_tensor(
                        qT_sb[:, h, :], psq[:], osq_row[:], ALU.mult)
                else:
                    tmpq = pipe.tile([P, TPC], f32, tag="qtmp", bufs=2,
                                     name="qtmp")
                    nc.vector.tensor_tensor(tmpq[:], psq[:], osq_row[:], ALU.mult)
                    nc.vector.tensor_scalar(
                        qT_sb[:, h, :], tmpq[:], qb_sb[:, h:h + 1], None, ALU.add)

            # ---------- land gathered K/V (K first — scores need it) --------
            nc.sync.dma_start(
                osk_all[:].rearrange("p (s c) -> p s c", s=GRP),
                cck_out[:, KC_SC:KC_N].bitcast(f32)
                .rearrange("s (p c) -> p s c", p=P),
            )
            for oc in range(KVH):
                o0 = oc * P * TPC
                nc.sync.dma_start(
                    kT_sb[:, oc].rearrange("p (s t) -> p s t", t=TPC),
                    cck_out[:, o0:o0 + P * TPC].rearrange("s (p t) -> p s t",
                                                          p=P),
                )
        mid_cm.__exit__(None, None, None)


        # ---------- attention (AV-transposed, software-pipelined) ----------
        # Per head-pair group g: sc(g) = scores+exp (probs in two kc-halves),
        # av(g) = AV^T + denominator matmuls + divide, pst(g) = PE transpose
        # back token-major + Pool drain + LN stats. Emission order
        #   sc0 sc1 av0 sc2 pst0 av1 sc3 pst1 av2 pst2 av3 pst3
        # keeps the PE sequencer free of head-of-line waits: av(g) psum reuse
        # and pst(g)'s divide dependency are always covered by an interleaved
        # scores block.
        with (
            tc_.tile_pool(name="ps2", bufs=1, space="PSUM") as ps2,
            tc_.tile_pool(name="probsp", bufs=1) as probsp,
        ):
            HKC = NKC // 2
            probs_t = {}

            def sc_block(g):
                kv, hp = divmod(g, 2)
                h0 = kv * 4 + hp * 2
                halves = []
                for hh in range(2):
                    ph = probsp.tile([P, HKC, 2, TPC], bf16, tag="probs",
                                     bufs=3, name=f"probs{g}_{hh}")
                    halves.append(ph)
                    for kk in range(HKC):
                        kc = hh * HKC + kk
                        ps_s = ps2.tile([P, 2, TPC], f32, tag="scores",
                                        bufs=2, name="ps_s")
                        nc.tensor.matmul(
                            ps_s[:], kT_sb[:, kv, kc * P:(kc + 1) * P],
                            qT_sb[:, h0:h0 + 2, :], start=True, stop=True,
                        )
                        nc.scalar.activation(
                            ph[:, kk], ps_s[:],
                            AF.Exp, scale=osk_all[:, kc:kc + 1],
                        )
                probs_t[g] = halves

            def av_block(g):
                kv, hp = divmod(g, 2)
                av = ps2.tile([P, 2, TPC], f32, tag="av", bufs=1, name="av")
                dn = ps2.tile([P, 2, TPC], f32, tag="denom", bufs=1, name="dn")
                for kc in range(NKC):
                    ph = probs_t[g][kc // HKC]
                    kk = kc % HKC
                    for j in range(2):
                        nc.tensor.matmul(
                            av[:, j, :], v_sc[:, kc, kv * P:(kv + 1) * P],
                            ph[:, kk, j, :],
                            start=(kc == 0), stop=(kc == NKC - 1),
                        )
                    nc.tensor.matmul(
                        dn[:], em.ones_bf[:], ph[:, kk],
                        start=(kc == 0), stop=(kc == NKC - 1),
                    )
                rden_p = probsp.tile([P, 2, TPC], f32, tag="rden", bufs=2,
                                     name="rden")
                nc.vector.reciprocal(rden_p[:], dn[:])
                xT_p = probsp.tile([P, 2, TPC], f32, tag="xT", bufs=2,
                                   name="xT")
                for j in range(2):
                    nc.vector.tensor_tensor(
                        xT_p[:, j, :], av[:, j, :], rden_p[:, j, :], ALU.mult)
                return xT_p

            def pst_block(g, xT_p):
                kv, hp = divmod(g, 2)
                h0 = kv * 4 + hp * 2
                pst = ps2.tile([P, 2, TPC], f32, tag="scores", bufs=2,
                               name="pst")
                for j in range(2):
                    h = h0 + j
                    for tcc in range(NTC):
                        nc.tensor.matmul(
                            pst[:, j, tcc * P:(tcc + 1) * P],
                            xT_p[:, j, tcc * P:(tcc + 1) * P],
                            em.ident[:],
                            is_transpose=True, start=True, stop=True,
                        )
                        nc.vector.tensor_copy(
                            x_sb[:, tcc, h * P:(h + 1) * P],
                            pst[:, j, tcc * P:(tcc + 1) * P])
                scrA = pipe.tile([P, 2 * P], f32, tag="scrA", bufs=1,
                                 name="scrA")
                for tcc in range(NTC):
                    xg = x_sb[:, tcc, h0 * P:(h0 + 2) * P]
                    nc.vector.reduce_sum(
                        s1p[:, tcc, g:g + 1], xg, axis=AX.X)
                    nc.vector.tensor_tensor(scrA[:], xg, xg, ALU.mult)
                    nc.vector.reduce_sum(
                        ssqp[:, tcc, g:g + 1], scrA[:], axis=AX.X)

            sc_block(0)
            sc_block(1)
            # land gathered V here: the wait hint keeps the scheduler's
            # internal sim from ordering the scores behind the V collective
            # (it would then compress score waits onto the V DMA queue).
            # V was pre-scaled by its out-scale on the source core.
            with tc_.tile_wait_until(0.125):
                for s_ in range(GRP):
                    nc.sync.dma_start(
                        v_sc[:, s_ * NTC:(s_ + 1) * NTC, :],
                        ccv_out[s_].rearrange("(tc p o) -> p tc o",
                                              p=P, o=KVE),
                    )
            xt0 = av_block(0)
            sc_block(2)
            pst_block(0, xt0)
            xt1 = av_block(1)
            sc_block(3)
            pst_block(1, xt1)
            xt2_ = av_block(2)
            pst_block(2, xt2_)
            xt3 = av_block(3)
            pst_block(3, xt3)

        # ---------- layernorm + final quant + output projection ----------
        with (
            tc_.tile_pool(name="ps3", bufs=1, space="PSUM") as ps3,
            tc_.tile_pool(name="opool", bufs=1) as opool,
        ):
            wmo = em.weight_prep(wo_d, QH, swoT, "o", ps3, wpool=opool,
                                 pe_t=True, drain="dve")
            sm = small

            def st(tag):
                return sm.tile([P, NTC], dt.float32, tag=tag, name=tag)

            s1, ssql = st("s1_ln"), st("ssq_ln")
            nc.vector.reduce_sum(s1[:], s1p[:], axis=AX.X)
            nc.vector.reduce_sum(ssql[:], ssqp[:], axis=AX.X)
            mu, e2, m2, var, sd, rstd, nmu = (
                st("mu"), st("e2"), st("m2"), st("var"), st("sd"), st("rstd"),
                st("nmu"))
            ssq2, amax2 = st("ssq2"), st("amax2")
            u2, c2, amn2, osl, ra2, m1l = (
                st("u2"), st("c2"), st("amn2"), st("osl"), st("ra2"), st("m1l"))
            syl = st("syl")
            scr2 = pipe.tile([P, D], dt.bfloat16, tag="scr", bufs=1, name="scr2")

            if not zln:
                g_row = persist.tile([P, D], dt.float32, tag="g_row", name="g_row")
                nc.gpsimd.dma_start(g_row[:], g_d[:].to_broadcast((P, D)))
                b_row = persist.tile([P, D], dt.float32, tag="b_row", name="b_row")
                nc.gpsimd.dma_start(b_row[:], bl_d[:].to_broadcast((P, D)))
            if not zb:
                ob_row = persist.tile([P, D], dt.float32, tag="ob_row",
                                      name="ob_row")
                nc.gpsimd.dma_start(ob_row[:], bo_d[:].to_broadcast((P, D)))

            yv = y_d.rearrange("(tc p) o -> p tc o", p=P)
            # two-token-tile halves: out-proj of half 0 overlaps the ln/quant
            # chain of half 1
            for hf in range(2):
                hs = slice(2 * hf, 2 * hf + 2)
                nc.vector.tensor_scalar_mul(mu[:, hs], s1[:, hs], 1.0 / D)
                nc.vector.tensor_scalar_mul(e2[:, hs], ssql[:, hs], 1.0 / D)
                nc.vector.tensor_tensor(m2[:, hs], mu[:, hs], mu[:, hs], ALU.mult)
                nc.vector.tensor_tensor(var[:, hs], e2[:, hs], m2[:, hs],
                                        ALU.subtract)
                nc.scalar.activation(sd[:, hs], var[:, hs], AF.Sqrt,
                                     bias=em.eps_ln[:])
                nc.vector.reciprocal(rstd[:, hs], sd[:, hs])
                nc.vector.tensor_scalar_mul(nmu[:, hs], mu[:, hs], -1.0)

                lt2 = em.live.tile([P, 2, D], dt.float32, tag=f"xt2_{hf}",
                                   name=f"lt2_{hf}")
                lt_aps = []
                for i in range(2):
                    tcc = 2 * hf + i
                    nc.vector.tensor_scalar(
                        lt2[:, i, :], x_sb[:, tcc], nmu[:, tcc:tcc + 1],
                        rstd[:, tcc:tcc + 1], ALU.add, ALU.mult,
                    )
                    if not zln:
                        nc.vector.tensor_tensor(
                            lt2[:, i, :], lt2[:, i, :], g_row[:], ALU.mult)
                        nc.vector.tensor_tensor(
                            lt2[:, i, :], lt2[:, i, :], b_row[:], ALU.add)
                    lt_aps.append(lt2[:, i, :])

                # bitlinear quant of this half
                for i, lt in enumerate(lt_aps):
                    tcc = 2 * hf + i
                    nc.scalar.activation(
                        scr2[:], lt, AF.Square, accum_out=ssq2[:, tcc:tcc + 1])
                    nc.vector.tensor_reduce(
                        amax2[:, tcc:tcc + 1], lt, AX.X, ALU.max,
                        apply_absolute_value=True)
                nc.scalar.activation(u2[:, hs], ssq2[:, hs], AF.Sqrt,
                                     bias=em.eps_rms[:])
                nc.vector.reciprocal(c2[:, hs], u2[:, hs])
                nc.vector.tensor_tensor(amn2[:, hs], c2[:, hs], amax2[:, hs],
                                        ALU.mult)
                nc.vector.tensor_scalar_max(amn2[:, hs], amn2[:, hs], 1e-5)
                nc.vector.tensor_scalar_mul(osl[:, hs], amn2[:, hs], 1.0 / 127.0)
                nc.vector.reciprocal(ra2[:, hs], amn2[:, hs])
                nc.vector.tensor_tensor(m1l[:, hs], c2[:, hs], ra2[:, hs],
                                        ALU.mult)
                nc.vector.tensor_scalar_mul(m1l[:, hs], m1l[:, hs], 127.0)
                nc.vector.tensor_tensor(
                    syl[:, hs], osl[:, hs],
                    wmo[:, 0:1].to_broadcast((P, 2)), ALU.mult)
                for i, lt in enumerate(lt_aps):
                    tcc = 2 * hf + i
                    tr = pipe.tile([P, D], dt.float32, tag="tr", bufs=2,
                                   name="tr")
                    nc.scalar.activation(
                        tr[:], lt, AF.Copy, bias=MAGIC,
                        scale=m1l[:, tcc:tcc + 1])
                    g = pipe.tile([P, D], dt.bfloat16, tag="gtile", bufs=2,
                                  name="g")
                    nc.vector.tensor_scalar_sub(g[:], tr[:], MAGIC)
                    nc.sync.dma_start_transpose(GlnT[:, tcc], g[:])
                for i in range(2):
                    tcc = 2 * hf + i
                    for og in range(2):
                        psum = ps3.tile([P, TPC], dt.float32, tag="yproj",
                                        bufs=3, name="py")
                        for dc_ in range(DC):
                            nc.tensor.matmul(
                                psum[:], GlnT[:, tcc, dc_, :],
                                swoT[:, og * 4:(og + 1) * 4, dc_, :],
                                start=(dc_ == 0), stop=(dc_ == DC - 1),
                            )
                        yt = pipe.tile([P, TPC], dt.float32, tag="yt", bufs=2,
                                       name="yt")
                        nc.vector.tensor_scalar(
                            yt[:], psum[:], syl[:, tcc:tcc + 1], None, ALU.mult)
                        if not zb:
                            nc.vector.tensor_tensor(
                                yt[:], yt[:],
                                ob_row[:, og * TPC:(og + 1) * TPC], ALU.add)
                        nc.sync.dma_start(
                            yv[:, tcc, og * TPC:(og + 1) * TPC], yt[:])

    _split_multiwaits(nc)
    return nc


def kernel(**inputs):
    from concourse.bass_utils import run_bass_kernel_spmd

    def arr(name):
        return np.ascontiguousarray(np.asarray(inputs[name], dtype=np.float32))

    q, k, v = arr("query"), arr("key"), arr("value")
    qw, kw, vw, ow = arr("q_w"), arr("k_w"), arr("v_w"), arr("out_w")
    qb, kb, vb, ob = arr("q_b"), arr("k_b"), arr("v_b"), arr("out_b")
    lg, lb = arr("ln_g"), arr("ln_b")

    zb = not (qb.any() or kb.any() or vb.any() or ob.any())
    zln = bool(np.all(lg == 1.0)) and not lb.any()

    key = (zb, zln)
    if key not in _BUILT:
        _BUILT[key] = build_nc(zb, zln)
    nc = _BUILT[key]

    qf = q.reshape(B * S, D)
    kf = k.reshape(B * S, D)
    vf = v.reshape(B * S, D)
    in_maps = []
    for c in range(N_CORES):
        sl = slice(c * TPC, (c + 1) * TPC)
        m = {
            "xq": qf[sl], "xk": kf[sl], "xv": vf[sl],
            "wq": qw, "wk": kw, "wv": vw, "wo": ow,
            "ident": np.eye(P, dtype=np.float32),
        }
        if not zb:
            m["bq"] = qb.reshape(1, D)
            m["bk"] = kb.reshape(1, KVE)
            m["bv"] = vb.reshape(1, KVE)
            m["bo"] = ob.reshape(1, D)
        if not zln:
            m["g_ln"] = lg.reshape(1, D)
            m["b_ln"] = lb.reshape(1, D)
        in_maps.append(m)

    res = run_bass_kernel_spmd(nc, in_maps, core_ids=list(range(N_CORES)))
    y = np.concatenate([res.results[c]["y"] for c in range(N_CORES)], axis=0)
    return y.reshape(B, S, D).astype(np.float32)


# revision 63
# speedup vs baseline: 1.0392x; 1.0237x over previous
"""BitMGQA forward for Trainium2, 8-core SPMD Bass/Tile kernel.

Sharding: the B*S = 4096 tokens are split into 8 slices of 512 (cores 0-3 =
batch 0, cores 4-7 = batch 1). Each core quantizes + projects its own token
slice for Q/K/V; the per-batch K/V (plus their per-token output scales) are
AllGathered across each 4-core batch group in one merged collective;
attention, layernorm and the output projection are then fully local to each
core's 512 query tokens.

BitLinear structure is exploited: activation quantization produces integers in
[-128, 127] and weight binarization produces +-1, both exactly representable
in bf16, so every projection matmul runs on the PE in bf16 with fp32 PSUM
accumulation exactly; rmsnorm / quant / weight scales fold into one per-token
fp32 scale applied to the matmul output (or, for K, into the softmax exp's
per-key scale operand).

Attention runs "AV-transposed": scores stay key-major ([key, token] psum,
exp'd in place with the per-key scale), and AV contracts keys with V as the
128-wide stationary so the moving dim is 512 tokens (big frees, few
instructions). Softmax denominators come from an all-ones stationary matmul
over the same probs. The [head_dim, token] attention output is divided by the
denominator, transposed back token-major on the PE (fp32-exact) with Pool
draining PSUM, and the layernorm/quant/out-proj tail runs token-major.

Scheduling: only the K/V path runs before the AllGather; Q quant/projection,
q/o weight prep and the *local* quarter of scores+exp fill the collective
window.
"""
import contextlib

import numpy as np

import concourse.bass as bass
import concourse.mybir as mybir
import concourse.tile as tile

dt = mybir.dt
AF = mybir.ActivationFunctionType
ALU = mybir.AluOpType
AX = mybir.AxisListType

# problem dims (hardcoded per contract)
B, S, D = 2, 2048, 1024
N_CORES = 8
GRP = 4                    # cores per batch group
TPC = (B * S) // N_CORES   # 512 tokens per core
P = 128
NTC = TPC // P             # 4 token tiles per core
DC = D // P                # 8 contraction chunks
KVE = 256
QH, KVH, HD = 8, 2, 128
NKC = S // P               # 16 key chunks per batch
LKC = TPC // P             # 4 key chunks produced locally
MAGIC = 12582912.0         # 1.5 * 2**23: (x + MAGIC) - MAGIC == rint(x)
INV_SQRT_HD = float(HD) ** -0.5
LN_EPS = 1e-5
RMS_EPS = 1e-6

_BUILT = {}


def _split_multiwaits(nc, max_waits=1):
    """The pinned walrus rejects >1 sync-wait per instruction ("Too many sync
    wait commands"). Split extras onto single-wait NoOps inserted before the
    offending instruction on the same engine (sequencer stalls in order, so
    semantics are identical)."""
    for f in nc.m.functions:
        for bb in f.blocks:
            insts = list(bb.instructions)
            if not any(
                i.sync_info is not None and len(i.sync_info.on_wait) > max_waits
                for i in insts
            ):
                continue
            new_insts = []
            for ins in insts:
                si = ins.sync_info
                if si is not None and len(si.on_wait) > max_waits:
                    waits = list(si.on_wait)
                    for k, w in enumerate(waits[:-max_waits]):
                        new_insts.append(mybir.InstNoOp(
                            name=f"{ins.name}-wsplit{k}",
                            engine=ins.engine,
                            sync_info=mybir.SyncInfo(on_wait=[w], on_update=[]),
                        ))
                    ins.sync_info = mybir.SyncInfo(
                        on_wait=waits[-max_waits:], on_update=list(si.on_update)
                    )
                new_insts.append(ins)
            bb.instructions = new_insts


class _Emit:
    """Per-build emission state."""

    def __init__(self, nc, tc_, ctx):
        self.nc = nc
        self.tc = tc_
        f32 = dt.float32
        self.small = ctx.enter_context(tc_.tile_pool(name="small", bufs=1))
        self.persist = ctx.enter_context(tc_.tile_pool(name="persist", bufs=1))
        self.pipe = ctx.enter_context(tc_.tile_pool(name="pipe", bufs=3))
        self.live = ctx.enter_context(tc_.tile_pool(name="live", bufs=1))
        self.dram = ctx.enter_context(
            tc_.tile_pool(name="dram", bufs=1, space="DRAM"))
        self.ones = self.small.tile([P, P], f32, tag="ones128", name="ones")
        nc.vector.memset(self.ones[:], 1.0)
        self.ones_bf = self.small.tile([P, P], dt.bfloat16, tag="ones128b",
                                       name="ones_bf")
        nc.vector.memset(self.ones_bf[:], 1.0)
        self.eps_rms = self.small.tile([P, 1], f32, tag="eps_rms", name="eps_rms")
        nc.vector.memset(self.eps_rms[:], float(D * RMS_EPS))
        self.eps_ln = self.small.tile([P, 1], f32, tag="eps_ln", name="eps_ln")
        nc.vector.memset(self.eps_ln[:], LN_EPS)
        # identity for PE transposes, loaded from an external input (the
        # gpsimd affine_select path is rejected by this walrus build);
        # build_nc fills these right after construction
        self.ident = self.small.tile([P, P], f32, tag="ident", name="ident")
        self.ident_bf = self.small.tile([P, P], dt.bfloat16, tag="ident_bf",
                                        name="ident_bf")

    # ---- helpers -------------------------------------------------------
    def weight_prep(self, w_dram, n_oc, swT, name, psred, wpool=None,
                    load_eng=None, pe_t=False, wait_ms=None, drain="act"):
        """sign(w)^T into swT [P, n_oc, DC, P] bf16; returns mean|w| as a
        [P, 1] fp32 column replicated across partitions. Loads weight rows
        two 128-row chunks at a time to halve DMA count. pe_t: transpose on
        the PE (Pool drains PSUM) instead of the DMA xbar — required after
        the collective is issued, since DmaTranspose serializes with it."""
        nc = self.nc
        wpool = wpool or self.wpool
        npair = n_oc // 2
        abscol = self.small.tile(
            [P, npair], dt.float32, tag=f"abscol_{name}", name=f"abscol_{name}")
        wv = w_dram.rearrange("(oc p) d -> p oc d", p=P)
        for pr in range(npair):
            wt = wpool.tile([P, 2, D], dt.float32, tag="wtile", bufs=3,
                            name="wt")
            with self.tc.tile_wait_until(wait_ms or 0, enable=wait_ms is not None):
                (load_eng or nc.sync).dma_start(wt[:], wv[:, 2 * pr:2 * pr + 2])
            sw = wpool.tile([P, 2, D], dt.bfloat16, tag="swtile", bufs=2,
                            name="sw")
            nc.scalar.sign(sw[:], wt[:])
            nc.vector.tensor_reduce(
                abscol[:, pr:pr + 1], wt[:], AX.XY, ALU.add,
                apply_absolute_value=True,
            )
            if pe_t:
                pswt = psred.tile([P, 2, DC, P], dt.bfloat16, tag="wtp",
                                  bufs=1, name="pswt")
                for i in range(2):
                    for dc_ in range(DC):
                        nc.tensor.matmul(
                            pswt[:, i, dc_, :], sw[:, i, dc_ * P:(dc_ + 1) * P],
                            self.ident_bf[:], is_transpose=True,
                            start=True, stop=True)
                    if drain == "act":
                        nc.scalar.copy(swT[:, 2 * pr + i], pswt[:, i])
                    else:
                        nc.vector.tensor_copy(swT[:, 2 * pr + i], pswt[:, i])
            else:
                for i in range(2):
                    nc.sync.dma_start_transpose(swT[:, 2 * pr + i], sw[:, i])
        rowtot = self.small.tile(
            [P, 1], dt.float32, tag=f"rowtot_{name}", name=f"rowtot_{name}")
        nc.vector.reduce_sum(rowtot[:], abscol[:], axis=AX.X)
        ps = psred.tile([P, 1], dt.float32, tag="psred", bufs=1, name="psred")
        nc.tensor.matmul(ps[:], self.ones[:], rowtot[:], start=True, stop=True)
        wm = self.small.tile([P, 1], dt.float32, tag=f"wm_{name}", name=f"wm_{name}")
        nc.scalar.mul(wm[:], ps[:], 1.0 / (n_oc * P * D))
        return wm

    def load_x(self, x_dram, tagpfx, tagbase="xt2", wait_ms=None, pool=None):
        """Load a [TPC, D] activation as two [P, 2, D] tiles; return the four
        [P, D] per-token-tile APs (token t = tc*128 + p)."""
        xv = x_dram.rearrange("(h tc p) d -> p h tc d", p=P, h=2)
        aps = []
        for h in range(2):
            xt = (pool or self.live).tile([P, 2, D], dt.float32,
                                          tag=f"{tagbase}_{h}",
                                          name=f"{tagpfx}{h}")
            with self.tc.tile_wait_until(wait_ms or 0,
                                         enable=wait_ms is not None):
                self.nc.sync.dma_start(xt[:], xv[:, h])
            aps.extend([xt[:, 0, :], xt[:, 1, :]])
        return aps

    def quant(self, specs, pe_t=False, psp=None, ssq_dve=False, drain="act"):
        """specs: list of (x_aps, GT, name). Quantize several tensors with
        ACT ops grouped by function (fewer activation-table switches).
        Returns dict name -> os [P, NTC] raw out-scale. pe_t: transpose the
        rounded ints on the PE instead of the DMA xbar (psp = psum pool).
        ssq_dve: compute the square-sums on DVE (tensor_tensor_reduce)
        instead of ACT, freeing the ACT queue for the critical path."""
        nc = self.nc
        sm = self.small

        def st(tag, name):
            return sm.tile([P, NTC], dt.float32, tag=f"{tag}_{name}",
                           name=f"{tag}_{name}")

        scr = self.pipe.tile([P, D], dt.bfloat16, tag="scr", bufs=1, name="scr")
        out = {}
        for x_aps, GT, name in specs:
            ssq, amax = st("ssq", name), st("amax", name)
            u, c, amn, os, ra, m1 = (st("u", name), st("c", name),
                                     st("amn", name), st("os", name),
                                     st("ra", name), st("m1", name))
            for tc in range(NTC):
                cs = slice(tc, tc + 1)
                if ssq_dve:
                    nc.vector.tensor_tensor_reduce(
                        scr[:], x_aps[tc], x_aps[tc], 1.0, 0.0,
                        ALU.mult, ALU.add, ssq[:, cs])
                else:
                    nc.scalar.activation(
                        scr[:], x_aps[tc], AF.Square, accum_out=ssq[:, cs])
                nc.vector.tensor_reduce(
                    amax[:, cs], x_aps[tc], AX.X, ALU.max,
                    apply_absolute_value=True,
                )
                nc.scalar.activation(u[:, cs], ssq[:, cs], AF.Sqrt,
                                     bias=self.eps_rms[:])
                nc.vector.reciprocal(c[:, cs], u[:, cs])
                nc.vector.tensor_tensor(amn[:, cs], c[:, cs], amax[:, cs],
                                        ALU.mult)
                nc.vector.tensor_scalar_max(amn[:, cs], amn[:, cs], 1e-5)
                nc.vector.tensor_scalar_mul(os[:, cs], amn[:, cs], 1.0 / 127.0)
                nc.vector.reciprocal(ra[:, cs], amn[:, cs])
                nc.vector.tensor_tensor(m1[:, cs], c[:, cs], ra[:, cs],
                                        ALU.mult)
                nc.vector.tensor_scalar_mul(m1[:, cs], m1[:, cs], 127.0)
                tr = self.pipe.tile([P, D], dt.float32, tag="tr", bufs=2,
                                    name="tr")
                nc.scalar.activation(
                    tr[:], x_aps[tc], AF.Copy, bias=MAGIC, scale=m1[:, cs])
                g = self.pipe.tile([P, D], dt.bfloat16, tag="gtile", bufs=2,
                                   name="g")
                nc.vector.tensor_scalar_sub(g[:], tr[:], MAGIC)
                if pe_t:
                    psg = psp.tile([P, DC, P], dt.bfloat16, tag="gtp",
                                   bufs=1, name="psg")
                    for dc_ in range(DC):
                        nc.tensor.matmul(
                            psg[:, dc_, :], g[:, dc_ * P:(dc_ + 1) * P],
                            self.ident_bf[:], is_transpose=True,
                            start=True, stop=True)
                    if drain == "act":
                        nc.scalar.copy(GT[:, tc], psg[:])
                    else:
                        nc.vector.tensor_copy(GT[:, tc], psg[:])
                else:
                    nc.sync.dma_start_transpose(GT[:, tc], g[:])
            out[name] = os
        return out

    def os_row(self, os_col, name):
        """[P, NTC] per-token column (t = tc*128+p) -> [P, TPC] fp32
        broadcast row via a DRAM bounce."""
        nc = self.nc
        scratch = self.dram.tile([1, TPC], dt.float32, tag=f"osrow_d_{name}",
                                 name=f"osrow_d_{name}")
        nc.gpsimd.dma_start(scratch[0].rearrange("(c p) -> p c", p=P), os_col[:])
        row = self.small.tile([P, TPC], dt.float32, tag=f"osrow_{name}",
                              name=f"osrow_{name}")
        nc.gpsimd.dma_start(row[:], scratch[:].to_broadcast((P, TPC)))
        return row

    def mul_wm(self, os, wm, name, extra=None):
        out = self.small.tile([P, NTC], dt.float32, tag=f"oss_{name}",
                              name=f"oss_{name}")
        self.nc.vector.tensor_tensor(
            out[:], os[:], wm[:, 0:1].to_broadcast((P, NTC)), ALU.mult)
        if extra is not None:
            self.nc.vector.tensor_scalar_mul(out[:], out[:], extra)
        return out


def build_nc(zb: bool, zln: bool):
    """zb: all projection biases zero; zln: ln_g == 1 and ln_b == 0."""
    nc = bass.Bass()
    f32, bf16 = dt.float32, dt.bfloat16

    xq_d = nc.dram_tensor("xq", [TPC, D], f32, kind="ExternalInput")
    xk_d = nc.dram_tensor("xk", [TPC, D], f32, kind="ExternalInput")
    xv_d = nc.dram_tensor("xv", [TPC, D], f32, kind="ExternalInput")
    wq_d = nc.dram_tensor("wq", [D, D], f32, kind="ExternalInput")
    wk_d = nc.dram_tensor("wk", [KVE, D], f32, kind="ExternalInput")
    wv_d = nc.dram_tensor("wv", [KVE, D], f32, kind="ExternalInput")
    wo_d = nc.dram_tensor("wo", [D, D], f32, kind="ExternalInput")
    if not zb:
        bq_d = nc.dram_tensor("bq", [1, D], f32, kind="ExternalInput")
        bk_d = nc.dram_tensor("bk", [1, KVE], f32, kind="ExternalInput")
        bv_d = nc.dram_tensor("bv", [1, KVE], f32, kind="ExternalInput")
        bo_d = nc.dram_tensor("bo", [1, D], f32, kind="ExternalInput")
    if not zln:
        g_d = nc.dram_tensor("g_ln", [1, D], f32, kind="ExternalInput")
        bl_d = nc.dram_tensor("b_ln", [1, D], f32, kind="ExternalInput")
    ident_d = nc.dram_tensor("ident", [P, P], f32, kind="ExternalInput")
    y_d = nc.dram_tensor("y", [TPC, D], f32, kind="ExternalOutput")

    groups = [[0, 1, 2, 3], [4, 5, 6, 7]]

    with tile.TileContext(nc) as tc_, contextlib.ExitStack() as ctx:
        em = _Emit(nc, tc_, ctx)
        small, persist, pipe, dram = em.small, em.persist, em.pipe, em.dram
        nc.sync.dma_start(em.ident[:], ident_d[:, :])
        nc.vector.tensor_copy(em.ident_bf[:], em.ident[:])

        # persistent SBUF structures
        swoT = persist.tile([P, QH, DC, P], bf16, tag="swoT", name="swoT")
        swkT = persist.tile([P, KVH, DC, P], bf16, tag="swkT", name="swkT")
        swvT = persist.tile([P, KVH, DC, P], bf16, tag="swvT", name="swvT")
        GlnT = persist.tile([P, NTC, DC, P], bf16, tag="GlnT", name="GlnT")
        # Q^T: [head_dim partition, head, token]
        qT_sb = persist.tile([P, QH, TPC], bf16, tag="qT_sb", name="qT_sb")
        kT_sb = persist.tile([P, KVH, S], bf16, tag="kT_sb", name="kT_sb")
        # V scaled by its per-source-token out-scale, key-major:
        # [key%128, key chunk, vdim (kv*128+d)]
        v_sc = persist.tile([P, NKC, KVE], bf16, tag="v_sc", name="v_sc")
        x_sb = persist.tile([P, NTC, D], f32, tag="x_sb", name="x_sb")
        # per-key exp scales, gathered with K
        osk_all = persist.tile([P, NKC], f32, tag="osk_all", name="osk_all")
        # incremental layernorm stats, accumulated per attention head-group
        s1p = persist.tile([P, NTC, 4], f32, tag="s1p", name="s1p")
        ssqp = persist.tile([P, NTC, 4], f32, tag="ssqp", name="ssqp")

        mid_cm = tc_.tile_pool(name="mid", bufs=1)
        mid = mid_cm.__enter__()
        em.wpool = mid
        swqT = mid.tile([P, QH, DC, P], bf16, tag="swqT", name="swqT")
        GqT = mid.tile([P, NTC, DC, P], bf16, tag="GqT", name="GqT")
        GkT = mid.tile([P, NTC, DC, P], bf16, tag="GkT", name="GkT")
        GvT = mid.tile([P, NTC, DC, P], bf16, tag="GvT", name="GvT")

        # two pipelined collectives: K (ints + key scales) fires first so
        # scores can start while V is still on the wire.
        #   K buffer: [ k ints (oc p t) | osk (p c) f32 ]
        #   V buffer: [ v ints (tc p o) | osv (p c) f32 ]
        KC_SC = KVE * TPC
        KC_N = KC_SC + 2 * TPC
        cck_in = dram.tile([KC_N], bf16, tag="cck_in", name="cck_in")
        cck_out = dram.tile([GRP, KC_N], bf16, tag="cck_out", name="cck_out")
        VC_SC = TPC * KVE
        ccv_in = dram.tile([VC_SC], bf16, tag="ccv_in", name="ccv_in")
        ccv_out = dram.tile([GRP, VC_SC], bf16, tag="ccv_out", name="ccv_out")

        with tc_.tile_pool(name="ps1", bufs=1, space="PSUM") as ps1:
            # ---------- K path first: load, quantize, project, scatter ------
            # (k weight load leads the DMA queue so sign(w_k) never blocks
            # the ACT queue ahead of the quant rounds)
            wmk = em.weight_prep(wk_d, KVH, swkT, "k", ps1, pe_t=True)
            xk_t = em.load_x(xk_d, "xk")
            xv_t = em.load_x(xv_d, "xv", tagbase="xv2", pool=mid)

            onecol = None
            if not zb:
                onecol = small.tile([P, NTC], f32, tag="onecol", name="onecol")
                nc.vector.memset(onecol[:], 1.0)

            oss_k = em.quant([(xk_t, GkT, "k")], pe_t=True, psp=ps1)
            osk_s = em.mul_wm(oss_k["k"], wmk, "k")
            nc.gpsimd.dma_start(
                cck_in[KC_SC:KC_N].bitcast(f32).rearrange("(p c) -> p c", p=P),
                osk_s[:] if zb else onecol[:])

            kT_loc = mid.tile([P, KVH, TPC], bf16, tag="kT_loc", name="kT_loc")
            v_loc = mid.tile([P, NTC, KVE], bf16, tag="v_loc", name="v_loc")
            if not zb:
                bk_sb = small.tile([P, KVH], f32, tag="bk_sb", name="bk_sb")
                nc.sync.dma_start(bk_sb[:], bk_d[0].rearrange("(c p) -> p c", p=P))
                oskb_row = em.os_row(osk_s, "oskb")
                vb_row = small.tile([P, KVE], f32, tag="vb_row", name="vb_row")
                nc.gpsimd.dma_start(vb_row[:], bv_d[:].to_broadcast((P, KVE)))
            # K: [o, t] orientation
            for oc in range(KVH):
                psum = ps1.tile([P, TPC], f32, tag="proj", bufs=2, name="pj")
                for dc_ in range(DC):
                    nc.tensor.matmul(
                        psum[:], swkT[:, oc, dc_, :], GkT[:, :, dc_, :],
                        start=(dc_ == 0), stop=(dc_ == DC - 1),
                    )
                if zb:
                    nc.vector.tensor_copy(kT_loc[:, oc], psum[:])
                else:
                    tmp = pipe.tile([P, TPC], f32, tag="kvtmp", bufs=2,
                                    name="kvtmp")
                    nc.vector.tensor_tensor(tmp[:], psum[:], oskb_row[:], ALU.mult)
                    nc.vector.tensor_scalar(
                        kT_loc[:, oc], tmp[:], bk_sb[:, oc:oc + 1], None, ALU.add)
            nc.gpsimd.dma_start(
                cck_in[0:KC_SC].rearrange("(oc p t) -> p oc t", p=P, t=TPC),
                kT_loc[:])
            nc.gpsimd.collective_compute(
                "AllGather", ALU.bypass, replica_groups=groups,
                ins=[cck_in.opt()], outs=[cck_out.opt()])

            # ---------- V path (square-sums on DVE to unclog ACT) -----------
            wmv = em.weight_prep(wv_d, KVH, swvT, "v", ps1, pe_t=True)
            oss_v = em.quant([(xv_t, GvT, "v")], pe_t=True, psp=ps1)
            osv_s = em.mul_wm(oss_v["v"], wmv, "v")
            # V: token-major [t, o] orientation, straight into the cc layout
            for tcc in range(NTC):
                psum = ps1.tile([P, KVE], f32, tag="projv", bufs=2, name="pv")
                for dc_ in range(DC):
                    nc.tensor.matmul(
                        psum[:], GvT[:, tcc, dc_, :],
                        swvT[:, :, dc_, :],
                        start=(dc_ == 0), stop=(dc_ == DC - 1),
                    )
                if zb:
                    nc.vector.tensor_scalar(
                        v_loc[:, tcc], psum[:], osv_s[:, tcc:tcc + 1], None,
                        ALU.mult)
                else:
                    tmp2 = pipe.tile([P, KVE], f32, tag="vtmp", bufs=2,
                                     name="vtmp")
                    nc.vector.tensor_scalar(
                        tmp2[:], psum[:], osv_s[:, tcc:tcc + 1], None, ALU.mult)
                    nc.vector.tensor_tensor(
                        v_loc[:, tcc], tmp2[:], vb_row[:], ALU.add)
            nc.gpsimd.dma_start(
                ccv_in[:].rearrange("(tc p o) -> p tc o", p=P, o=KVE),
                v_loc[:])
            nc.gpsimd.collective_compute(
                "AllGather", ALU.bypass, replica_groups=groups,
                ins=[ccv_in.opt()], outs=[ccv_out.opt()])

            # ---------- overlaps collective: q/o weights + Q^T ----------
            # (everything here avoids DmaTranspose, which would serialize
            # against the in-flight collective)
            xq_t = em.load_x(xq_d, "xq", wait_ms=0.034)
            wmq = em.weight_prep(wq_d, QH, swqT, "q", ps1, pe_t=True,
                                 wait_ms=0.036, drain="dve")
            osq = em.quant([(xq_t, GqT, "q")], pe_t=True, psp=ps1,
                           drain="dve")["q"]
            osq_s = em.mul_wm(osq, wmq, "q", extra=INV_SQRT_HD)
            osq_row = em.os_row(osq_s, "osq")
            if not zb:
                qb_sb = small.tile([P, QH], f32, tag="qb_sb", name="qb_sb")
                nc.sync.dma_start(qb_sb[:], bq_d[0].rearrange("(c p) -> p c", p=P))
                # reference scales q (incl. bias) by 1/sqrt(hd)
                nc.vector.tensor_scalar_mul(qb_sb[:], qb_sb[:], INV_SQRT_HD)
            # Q^T directly: per head, out = [head_dim, token]
            for h in range(QH):
                psq = ps1.tile([P, TPC], f32, tag="proj", bufs=2, name="pq")
                for dc_ in range(DC):
                    nc.tensor.matmul(
                        psq[:], swqT[:, h, dc_, :], GqT[:, :, dc_, :],
                        start=(dc_ == 0), stop=(dc_ == DC - 1),
                    )
                if zb:
                    nc.vector.tensor_tensor(
                        qT_sb[:, h, :], psq[:], osq_row[:], ALU.mult)
                else:
                    tmpq = pipe.tile([P, TPC], f32, tag="qtmp", bufs=2,
                                     name="qtmp")
                    nc.vector.tensor_tensor(tmpq[:], psq[:], osq_row[:], ALU.mult)
                    nc.vector.tensor_scalar(
                        qT_sb[:, h, :], tmpq[:], qb_sb[:, h:h + 1], None, ALU.add)

            # ---------- land gathered K/V (K first — scores need it) --------
            nc.sync.dma_start(
                osk_all[:].rearrange("p (s c) -> p s c", s=GRP),
                cck_out[:, KC_SC:KC_N].bitcast(f32)
                .rearrange("s (p c) -> p s c", p=P),
            )
            for oc in range(KVH):
                o0 = oc * P * TPC
                nc.sync.dma_start(
                    kT_sb[:, oc].rearrange("p (s t) -> p s t", t=TPC),
                    cck_out[:, o0:o0 + P * TPC].rearrange("s (p t) -> p s t",
                                                          p=P),
                )
        mid_cm.__exit__(None, None, None)


        # ---------- attention (AV-transposed, software-pipelined) ----------
        # Per head-pair group g: sc(g) = scores+exp (probs in two kc-halves),
        # av(g) = AV^T + denominator matmuls + divide, pst(g) = PE transpose
        # back token-major + Pool drain + LN stats. Emission order
        #   sc0 sc1 av0 sc2 pst0 av1 sc3 pst1 av2 pst2 av3 pst3
        # keeps the PE sequencer free of head-of-line waits: av(g) psum reuse
        # and pst(g)'s divide dependency are always covered by an interleaved
        # scores block.
        with (
            tc_.tile_pool(name="ps2", bufs=1, space="PSUM") as ps2,
            tc_.tile_pool(name="probsp", bufs=1) as probsp,
        ):
            HKC = NKC // 2
            probs_t = {}

            def sc_block(g):
                kv, hp = divmod(g, 2)
                h0 = kv * 4 + hp * 2
                halves = []
                for hh in range(2):
                    ph = probsp.tile([P, HKC, 2, TPC], bf16, tag="probs",
                                     bufs=3, name=f"probs{g}_{hh}")
                    halves.append(ph)
                    for kk in range(HKC):
                        kc = hh * HKC + kk
                        ps_s = ps2.tile([P, 2, TPC], f32, tag="scores",
                                        bufs=2, name="ps_s")
                        for j in range(2):
                            nc.tensor.matmul(
                                ps_s[:, j, :], kT_sb[:, kv, kc * P:(kc + 1) * P],
                                qT_sb[:, h0 + j, :], start=True, stop=True,
                            )
                        nc.scalar.activation(
                            ph[:, kk], ps_s[:],
                            AF.Exp, scale=osk_all[:, kc:kc + 1],
                        )
                probs_t[g] = halves

            def av_block(g):
                kv, hp = divmod(g, 2)
                av = ps2.tile([P, 2, TPC], f32, tag="av", bufs=1, name="av")
                dn = ps2.tile([P, 2, TPC], f32, tag="denom", bufs=1, name="dn")
                # DVE pre-sums adjacent key-chunk prob pairs; the
                # denominator matmuls then stream half as many rows
                pairs = probsp.tile([P, HKC, 2, TPC], bf16, tag="pairs",
                                    bufs=1, name="pairs")
                for kc in range(NKC):
                    ph = probs_t[g][kc // HKC]
                    kk = kc % HKC
                    for j in range(2):
                        nc.tensor.matmul(
                            av[:, j, :], v_sc[:, kc, kv * P:(kv + 1) * P],
                            ph[:, kk, j, :],
                            start=(kc == 0), stop=(kc == NKC - 1),
                        )
                    if kc % 2 == 1:
                        pk = kc // 2
                        nc.vector.tensor_tensor(
                            pairs[:, pk], ph[:, kk - 1], ph[:, kk], ALU.add)
                        for j in range(2):
                            nc.tensor.matmul(
                                dn[:, j, :], em.ones_bf[:],
                                pairs[:, pk, j, :],
                                start=(pk == 0), stop=(pk == HKC // 2 - 1 if False else pk == HKC - 1),
                            )
                rden_p = probsp.tile([P, 2, TPC], f32, tag="rden", bufs=2,
                                     name="rden")
                nc.vector.reciprocal(rden_p[:], dn[:])
                xT_p = probsp.tile([P, 2, TPC], f32, tag="xT", bufs=2,
                                   name="xT")
                for j in range(2):
                    nc.vector.tensor_tensor(
                        xT_p[:, j, :], av[:, j, :], rden_p[:, j, :], ALU.mult)
                return xT_p

            def pst_block(g, xT_p):
                kv, hp = divmod(g, 2)
                h0 = kv * 4 + hp * 2
                pst = ps2.tile([P, 2, TPC], f32, tag="scores", bufs=2,
                               name="pst")
                for j in range(2):
                    h = h0 + j
                    for tcc in range(NTC):
                        nc.tensor.matmul(
                            pst[:, j, tcc * P:(tcc + 1) * P],
                            xT_p[:, j, tcc * P:(tcc + 1) * P],
                            em.ident[:],
                            is_transpose=True, start=True, stop=True,
                        )
                        nc.vector.tensor_copy(
                            x_sb[:, tcc, h * P:(h + 1) * P],
                            pst[:, j, tcc * P:(tcc + 1) * P])
                scrA = pipe.tile([P, 2 * P], f32, tag="scrA", bufs=1,
                                 name="scrA")
                for tcc in range(NTC):
                    xg = x_sb[:, tcc, h0 * P:(h0 + 2) * P]
                    nc.vector.reduce_sum(
                        s1p[:, tcc, g:g + 1], xg, axis=AX.X)
                    nc.vector.tensor_tensor(scrA[:], xg, xg, ALU.mult)
                    nc.vector.reduce_sum(
                        ssqp[:, tcc, g:g + 1], scrA[:], axis=AX.X)

            sc_block(0)
            sc_block(1)
            # land gathered V here: the wait hint keeps the scheduler's
            # internal sim from ordering the scores behind the V collective
            # (it would then compress score waits onto the V DMA queue).
            # V was pre-scaled by its out-scale on the source core.
            with tc_.tile_wait_until(0.125):
                for s_ in range(GRP):
                    nc.sync.dma_start(
                        v_sc[:, s_ * NTC:(s_ + 1) * NTC, :],
                        ccv_out[s_].rearrange("(tc p o) -> p tc o",
                                              p=P, o=KVE),
                    )
            xt0 = av_block(0)
            sc_block(2)
            pst_block(0, xt0)
            xt1 = av_block(1)
            sc_block(3)
            pst_block(1, xt1)
            xt2_ = av_block(2)
            pst_block(2, xt2_)
            xt3 = av_block(3)
            pst_block(3, xt3)

        # ---------- layernorm + final quant + output projection ----------
        with (
            tc_.tile_pool(name="ps3", bufs=1, space="PSUM") as ps3,
            tc_.tile_pool(name="opool", bufs=1) as opool,
        ):
            wmo = em.weight_prep(wo_d, QH, swoT, "o", ps3, wpool=opool,
                                 pe_t=True, drain="dve")
            sm = small

            def st(tag):
                return sm.tile([P, NTC], dt.float32, tag=tag, name=tag)

            s1, ssql = st("s1_ln"), st("ssq_ln")
            nc.vector.reduce_sum(s1[:], s1p[:], axis=AX.X)
            nc.vector.reduce_sum(ssql[:], ssqp[:], axis=AX.X)
            mu, e2, m2, var, sd, rstd, nmu = (
                st("mu"), st("e2"), st("m2"), st("var"), st("sd"), st("rstd"),
                st("nmu"))
            ssq2, amax2 = st("ssq2"), st("amax2")
            u2, c2, amn2, osl, ra2, m1l = (
                st("u2"), st("c2"), st("amn2"), st("osl"), st("ra2"), st("m1l"))
            syl = st("syl")
            scr2 = pipe.tile([P, D], dt.bfloat16, tag="scr", bufs=1, name="scr2")

            if not zln:
                g_row = persist.tile([P, D], dt.float32, tag="g_row", name="g_row")
                nc.gpsimd.dma_start(g_row[:], g_d[:].to_broadcast((P, D)))
                b_row = persist.tile([P, D], dt.float32, tag="b_row", name="b_row")
                nc.gpsimd.dma_start(b_row[:], bl_d[:].to_broadcast((P, D)))
            if not zb:
                ob_row = persist.tile([P, D], dt.float32, tag="ob_row",
                                      name="ob_row")
                nc.gpsimd.dma_start(ob_row[:], bo_d[:].to_broadcast((P, D)))

            yv = y_d.rearrange("(tc p) o -> p tc o", p=P)
            # two-token-tile halves: out-proj of half 0 overlaps the ln/quant
            # chain of half 1
            for hf in range(2):
                hs = slice(2 * hf, 2 * hf + 2)
                nc.vector.tensor_scalar_mul(mu[:, hs], s1[:, hs], 1.0 / D)
                nc.vector.tensor_scalar_mul(e2[:, hs], ssql[:, hs], 1.0 / D)
                nc.vector.tensor_tensor(m2[:, hs], mu[:, hs], mu[:, hs], ALU.mult)
                nc.vector.tensor_tensor(var[:, hs], e2[:, hs], m2[:, hs],
                                        ALU.subtract)
                nc.scalar.activation(sd[:, hs], var[:, hs], AF.Sqrt,
                                     bias=em.eps_ln[:])
                nc.vector.reciprocal(rstd[:, hs], sd[:, hs])
                nc.vector.tensor_scalar_mul(nmu[:, hs], mu[:, hs], -1.0)

                lt2 = em.live.tile([P, 2, D], dt.float32, tag=f"xt2_{hf}",
                                   name=f"lt2_{hf}")
                lt_aps = []
                for i in range(2):
                    tcc = 2 * hf + i
                    nc.vector.tensor_scalar(
                        lt2[:, i, :], x_sb[:, tcc], nmu[:, tcc:tcc + 1],
                        rstd[:, tcc:tcc + 1], ALU.add, ALU.mult,
                    )
                    if not zln:
                        nc.vector.tensor_tensor(
                            lt2[:, i, :], lt2[:, i, :], g_row[:], ALU.mult)
                        nc.vector.tensor_tensor(
                            lt2[:, i, :], lt2[:, i, :], b_row[:], ALU.add)
                    lt_aps.append(lt2[:, i, :])

                # bitlinear quant of this half
                for i, lt in enumerate(lt_aps):
                    tcc = 2 * hf + i
                    nc.scalar.activation(
                        scr2[:], lt, AF.Square, accum_out=ssq2[:, tcc:tcc + 1])
                    nc.vector.tensor_reduce(
                        amax2[:, tcc:tcc + 1], lt, AX.X, ALU.max,
                        apply_absolute_value=True)
                nc.scalar.activation(u2[:, hs], ssq2[:, hs], AF.Sqrt,
                                     bias=em.eps_rms[:])
                nc.vector.reciprocal(c2[:, hs], u2[:, hs])
                nc.vector.tensor_tensor(amn2[:, hs], c2[:, hs], amax2[:, hs],
                                        ALU.mult)
                nc.vector.tensor_scalar_max(amn2[:, hs], amn2[:, hs], 1e-5)
                nc.vector.tensor_scalar_mul(osl[:, hs], amn2[:, hs], 1.0 / 127.0)
                nc.vector.reciprocal(ra2[:, hs], amn2[:, hs])
                nc.vector.tensor_tensor(m1l[:, hs], c2[:, hs], ra2[:, hs],
                                        ALU.mult)
                nc.vector.tensor_scalar_mul(m1l[:, hs], m1l[:, hs], 127.0)
                nc.vector.tensor_tensor(
                    syl[:, hs], osl[:, hs],
                    wmo[:, 0:1].to_broadcast((P, 2)), ALU.mult)
                for i, lt in enumerate(lt_aps):
                    tcc = 2 * hf + i
                    tr = pipe.tile([P, D], dt.float32, tag="tr", bufs=2,
                                   name="tr")
                    nc.scalar.activation(
                        tr[:], lt, AF.Copy, bias=MAGIC,
                        scale=m1l[:, tcc:tcc + 1])
                    g = pipe.tile([P, D], dt.bfloat16, tag="gtile", bufs=2,
                                  name="g")
                    nc.vector.tensor_scalar_sub(g[:], tr[:], MAGIC)
                    nc.sync.dma_start_transpose(GlnT[:, tcc], g[:])
                for i in range(2):
                    tcc = 2 * hf + i
                    for og in range(2):
                        psum = ps3.tile([P, TPC], dt.float32, tag="yproj",
                                        bufs=3, name="py")
                        for dc_ in range(DC):
                            nc.tensor.matmul(
                                psum[:], GlnT[:, tcc, dc_, :],
                                swoT[:, og * 4:(og + 1) * 4, dc_, :],
                                start=(dc_ == 0), stop=(dc_ == DC - 1),
                            )
                        yt = pipe.tile([P, TPC], dt.float32, tag="yt", bufs=2,
                                       name="yt")
                        nc.vector.tensor_scalar(
                            yt[:], psum[:], syl[:, tcc:tcc + 1], None, ALU.mult)
                        if not zb:
                            nc.vector.tensor_tensor(
                                yt[:], yt[:],
                                ob_row[:, og * TPC:(og + 1) * TPC], ALU.add)
                        nc.sync.dma_start(
                            yv[:, tcc, og * TPC:(og + 1) * TPC], yt[:])

    _split_multiwaits(nc)
    return nc


def kernel(**inputs):
    from concourse.bass_utils import run_bass_kernel_spmd

    def arr(name):
        return np.ascontiguousarray(np.asarray(inputs[name], dtype=np.float32))

    q, k, v = arr("query"), arr("key"), arr("value")
    qw, kw, vw, ow = arr("q_w"), arr("k_w"), arr("v_w"), arr("out_w")
    qb, kb, vb, ob = arr("q_b"), arr("k_b"), arr("v_b"), arr("out_b")
    lg, lb = arr("ln_g"), arr("ln_b")

    zb = not (qb.any() or kb.any() or vb.any() or ob.any())
    zln = bool(np.all(lg == 1.0)) and not lb.any()

    key = (zb, zln)
    if key not in _BUILT:
        _BUILT[key] = build_nc(zb, zln)
    nc = _BUILT[key]

    qf = q.reshape(B * S, D)
    kf = k.reshape(B * S, D)
    vf = v.reshape(B * S, D)
    in_maps = []
    for c in range(N_CORES):
        sl = slice(c * TPC, (c + 1) * TPC)
        m = {
            "xq": qf[sl], "xk": kf[sl], "xv": vf[sl],
            "wq": qw, "wk": kw, "wv": vw, "wo": ow,
            "ident": np.eye(P, dtype=np.float32),
        }
        if not zb:
            m["bq"] = qb.reshape(1, D)
            m["bk"] = kb.reshape(1, KVE)
            m["bv"] = vb.reshape(1, KVE)
            m["bo"] = ob.reshape(1, D)
        if not zln:
            m["g_ln"] = lg.reshape(1, D)
            m["b_ln"] = lb.reshape(1, D)
        in_maps.append(m)

    res = run_bass_kernel_spmd(nc, in_maps, core_ids=list(range(N_CORES)))
    y = np.concatenate([res.results[c]["y"] for c in range(N_CORES)], axis=0)
    return y.reshape(B, S, D).astype(np.float32)
